# revision 10
# baseline (speedup 1.0000x reference)
"""Trainium2 Bass kernel for the CustomGNN message-passing network.

Strategy (node-parallel across 8 NeuronCores, no collectives needed):
  - `senders` is unused by the reference network and edge latents never
    change across the 10 MP steps, so the segment-sum aggregation is
    loop-invariant: compute it once.
  - seg_id = receiver*3 + type decomposes into 3 independent per-type
    segment sums.  Each core owns a contiguous block of N/8 nodes and
    processes exactly the edges whose receiver lands in its block, so the
    whole network (edge encoders, aggregation, node update loop, decoder)
    is embarrassingly parallel across cores.
  - On-device layout: activations are kept transposed ([feature, token]),
    which lets every linear run with stationary weights on the tensor
    engine.  The last linear of each MLP uses the activation tile as the
    stationary operand, producing token-major output so LayerNorm can use
    per-partition (per-token) statistics, then transposes back via the PE.
  - Per-type segment sums are computed as block matmuls: edges are sorted
    by receiver on the host and packed into groups of 64 segments with a
    fixed number of 128-edge tiles per group; a data-driven 0/1 selection
    matrix (built on-device from uploaded local indices via is_equal
    against an iota) maps edge latents to segment columns in PSUM.
"""

import math
import os
import sys
import types

for _p in ("/opt/trn_rl_repo",):
    if os.path.isdir(_p) and _p not in sys.path:
        sys.path.insert(0, _p)

import numpy as np

import concourse.bass as bass
import concourse.tile as tile
from concourse import bacc, mybir
from concourse.bass_utils import run_bass_kernel_spmd

F32 = mybir.dt.float32
F16 = mybir.dt.float16
AF = mybir.ActivationFunctionType
ALU = mybir.AluOpType

N_CORES = 8
GSEG = 64  # segments per aggregation group
EPS = 1e-5


def _np(a, dt=np.float32):
    return np.asarray(a).astype(dt)


def _mlp_arrays(mlp):
    Ws = [_np(W) for W in mlp["Ws"]]
    bs = [_np(b) for b in mlp["bs"]]
    g = _np(mlp["g"]) if mlp.get("g") is not None else None
    beta = _np(mlp["beta"]) if mlp.get("beta") is not None else None
    return Ws, bs, g, beta


def _col(v):
    return np.ascontiguousarray(v.reshape(-1, 1).astype(np.float32))


class Meta:
    pass


def prepare_host(inputs, n_cores=N_CORES):
    """Shard + sort edges, pack per-core device buffers, prep weights."""
    m = Meta()
    node_feats = _np(inputs["node_feats"])
    feats_by_type = [_np(inputs["body_feats"]), _np(inputs["cable_feats"]),
                     _np(inputs["con_feats"])]
    recv = np.asarray(inputs["receivers"]).astype(np.int64)
    N = node_feats.shape[0]
    assert N % n_cores == 0
    n_loc = N // n_cores
    tpad = ((n_loc + 127) // 128) * 128
    ngrp = tpad // GSEG
    m.N, m.n_loc, m.tpad, m.ngrp = N, n_loc, tpad, ngrp
    m.node_sz = node_feats.shape[1]
    m.esz = feats_by_type[0].shape[1]
    m.n_cores = n_cores

    ne = [f.shape[0] for f in feats_by_type]
    off = np.cumsum([0] + ne)
    # per (type): global sort by receiver once, then carve per-core ranges
    per_ct = [[None] * 3 for _ in range(n_cores)]
    maxcnt = 1
    for t in range(3):
        r_t = recv[off[t]:off[t + 1]]
        order = np.argsort(r_t, kind="stable")
        r_sorted = r_t[order]
        bounds = np.searchsorted(r_sorted, np.arange(0, N + 1, n_loc))
        for c in range(n_cores):
            sl = slice(bounds[c], bounds[c + 1])
            lr = r_sorted[sl] - c * n_loc
            idx = order[sl]
            grp = lr // GSEG
            cnt = np.bincount(grp, minlength=ngrp)
            maxcnt = max(maxcnt, int(cnt.max()) if len(cnt) else 1)
            per_ct[c][t] = (lr, idx, grp, cnt)
    t_g = (maxcnt + 127) // 128
    slots_g = t_g * 128
    e_slots = ngrp * slots_g
    m.t_g, m.slots_g, m.e_slots, m.nt = t_g, slots_g, e_slots, ngrp * t_g

    m.core_data = []
    for c in range(n_cores):
        d = {}
        for t in range(3):
            lr, idx, grp, cnt = per_ct[c][t]
            gstart = np.cumsum(cnt) - cnt
            rank = np.arange(len(lr)) - gstart[grp]
            slot = grp * slots_g + rank
            fT = np.zeros((m.esz, e_slots), np.float16)
            fT[:, slot] = feats_by_type[t][idx].T
            lidxb = np.full(e_slots, 127.0, np.float16)
            lidxb[slot] = (lr % GSEG).astype(np.float16)
            d[f"ef{t}"] = fT
            d[f"lidx{t}"] = np.ascontiguousarray(lidxb.reshape(m.nt, 128).T)
            d[f"cntseg{t}"] = np.bincount(lr, minlength=tpad).astype(np.float16)
        nT = np.zeros((m.node_sz, tpad), np.float16)
        nT[:, :n_loc] = node_feats[c * n_loc:(c + 1) * n_loc].T
        d["nodeT"] = nT
        m.core_data.append(d)

    # ---- weights ----
    w = {}
    enc_keys = ["body_enc", "cable_enc", "con_enc"]
    uWs, ubs, ug, ubeta = _mlp_arrays(inputs["node_upd"])
    L = uWs[-1].shape[0]
    m.L = L
    m.has_b3 = {}
    for t in range(3):
        Ws, bs, g, beta = _mlp_arrays(inputs[enc_keys[t]])
        w[f"enc{t}_w1T"] = Ws[0].T.astype(np.float16)
        w[f"enc{t}_w2T"] = Ws[1].T.astype(np.float16)
        w[f"enc{t}_w3T"] = Ws[2].T.astype(np.float16)
        w[f"enc{t}_b1"] = _col(bs[0])
        w[f"enc{t}_b2"] = _col(bs[1])
        m.has_b3[f"enc{t}"] = bool(np.any(bs[2]))
        if m.has_b3[f"enc{t}"]:
            w[f"enc{t}_b3rep"] = np.tile(bs[2].astype(np.float16), (128, 1))
        # fold edge-LN g into the pre-projection, beta via counts
        W1b_t = uWs[0][:, L * (t + 1):L * (t + 2)]
        w[f"preW{t}"] = (W1b_t.T * g[:, None]).astype(np.float16)
        vec = W1b_t @ beta
        if t == 0:
            m.beta_vecs = []
        m.beta_vecs.append(vec)
    m.has_ebeta = bool(any(np.any(v) for v in m.beta_vecs))
    if m.has_ebeta:
        w["betaW"] = np.stack(m.beta_vecs).astype(np.float16)  # [3, 128]

    nWs, nbs, ng, nbeta = _mlp_arrays(inputs["node_enc"])
    w["node_w1T"] = nWs[0].T.astype(np.float16)
    w["node_w2T"] = nWs[1].T.astype(np.float16)
    w["node_w3T"] = nWs[2].T.astype(np.float16)
    w["node_b1"] = _col(nbs[0])
    w["node_b2"] = _col(nbs[1])
    m.has_b3["node"] = bool(np.any(nbs[2]))
    if m.has_b3["node"]:
        w["node_b3rep"] = np.tile(nbs[2].astype(np.float16), (128, 1))
    w["node_g"] = _col(ng)
    w["node_beta"] = _col(nbeta)

    w["upd_w1aT"] = uWs[0][:, :L].T.astype(np.float16)
    w["upd_w2T"] = uWs[1].T.astype(np.float16)
    w["upd_w3T"] = uWs[2].T.astype(np.float16)
    w["upd_b1pre"] = _col(ubs[0])
    w["upd_b2"] = _col(ubs[1])
    m.has_b3["upd"] = bool(np.any(ubs[2]))
    if m.has_b3["upd"]:
        w["upd_b3rep"] = np.tile(ubs[2].astype(np.float16), (128, 1))
    w["upd_g"] = _col(ug)
    w["upd_beta"] = _col(ubeta)

    dWs, dbs, _, _ = _mlp_arrays(inputs["dec"])
    w["dec_w1T"] = dWs[0].T.astype(np.float16)
    w["dec_w2T"] = dWs[1].T.astype(np.float16)
    w["dec_w3T"] = dWs[2].T.astype(np.float16)
    w["dec_b1"] = _col(dbs[0])
    w["dec_b2"] = _col(dbs[1])
    w["dec_b3"] = _col(dbs[2])
    m.out_sz = dWs[2].shape[0]

    w["ident"] = np.eye(128, dtype=np.float16)
    w["iota4"] = np.tile(np.arange(GSEG, dtype=np.float16), (128, t_g, 1))
    m.weights = w
    return m


def _chunks(total, step=512):
    out = []
    c = 0
    while c < total:
        out.append((c, min(step, total - c)))
        c += step
    return out


def build_program(m):
    nc = bacc.Bacc("TRN2", target_bir_lowering=False, debug=False)
    D = {}

    def din(name, shape, dt):
        D[name] = nc.dram_tensor(name, list(shape), dt, kind="ExternalInput").ap()

    for t in range(3):
        din(f"ef{t}", (m.esz, m.e_slots), F16)
        din(f"lidx{t}", (128, m.nt), F16)
    if m.has_ebeta:
        din("cnt3", (3, m.tpad), F16)
    din("nodeT", (m.node_sz, m.tpad), F16)
    for k, v in m.weights.items():
        din(k, v.shape, F16 if v.dtype == np.float16 else F32)
    out_ap = nc.dram_tensor("outT", [m.out_sz, m.tpad], F32,
                            kind="ExternalOutput").ap()

    L = m.L
    from contextlib import ExitStack
    with tile.TileContext(nc) as tc, ExitStack() as ctx:
        sing = ctx.enter_context(tc.tile_pool(name="sing", bufs=1))
        big = ctx.enter_context(tc.tile_pool(name="big", bufs=1))
        fpool = ctx.enter_context(tc.tile_pool(name="fpool", bufs=3))
        hpool = ctx.enter_context(tc.tile_pool(name="hpool", bufs=3))
        latp = ctx.enter_context(tc.tile_pool(name="latp", bufs=4))
        selp = ctx.enter_context(tc.tile_pool(name="selp", bufs=3))
        stp = ctx.enter_context(tc.tile_pool(name="stp", bufs=8))
        outp = ctx.enter_context(tc.tile_pool(name="outp", bufs=3))
        pbig = ctx.enter_context(tc.tile_pool(name="pbig", bufs=2, space="PSUM"))
        ptm = ctx.enter_context(tc.tile_pool(name="ptm", bufs=2, space="PSUM"))
        ptr = ctx.enter_context(tc.tile_pool(name="ptr", bufs=2, space="PSUM"))
        pagg = ctx.enter_context(tc.tile_pool(name="pagg", bufs=2, space="PSUM"))

        # resident SBUF tensors
        S = {}
        for k, v in m.weights.items():
            dt = F16 if v.dtype == np.float16 else F32
            S[k] = sing.tile(list(v.shape), dt, name=k, tag=k)
            nc.sync.dma_start(out=S[k][:], in_=D[k][:])
        lidx_sb = []
        for t in range(3):
            lt = sing.tile([128, m.nt], F16, name=f"lidx{t}", tag=f"lidx{t}")
            nc.sync.dma_start(out=lt[:], in_=D[f"lidx{t}"][:])
            lidx_sb.append(lt)
        if m.has_ebeta:
            cnt_sb = sing.tile([3, m.tpad], F16)
            nc.sync.dma_start(out=cnt_sb[:], in_=D["cnt3"][:])
        nodeT_sb = sing.tile([m.node_sz, m.tpad], F16)
        nc.sync.dma_start(out=nodeT_sb[:], in_=D["nodeT"][:])
        eps_sb = sing.tile([128, 1], F32)
        nc.vector.memset(eps_sb[:], EPS)

        agg_sb = [big.tile([128, m.tpad], F16, tag=f"agg{t}", name=f"agg{t}") for t in range(3)]
        x_sb = [big.tile([128, m.tpad], F16, tag=f"x{i}", name=f"x{i}") for i in range(2)]
        pre_sb = big.tile([128, m.tpad], F32, tag="pre")

        ident = S["ident"]
        iota4 = S["iota4"]

        def layernorm_tile(z_psum, y_out, b3rep):
            """token-major z [128,128] psum -> normalized y (no g/beta)."""
            if b3rep is not None:
                nc.vector.tensor_tensor(out=z_psum[:], in0=z_psum[:],
                                        in1=b3rep[:], op=ALU.add)
            st6 = stp.tile([128, 6], F32, tag="st6", name="st6")
            nc.vector.bn_stats(out=st6[:], in_=z_psum[:])
            mv = stp.tile([128, 2], F32, tag="mv", name="mv")
            nc.vector.bn_aggr(out=mv[:], in_=st6[:])
            sd = stp.tile([128, 1], F32, tag="sd", name="sd")
            nc.scalar.activation(out=sd[:], in_=mv[:, 1:2], func=AF.Sqrt,
                                 bias=eps_sb[:], scale=1.0)
            nc.vector.reciprocal(out=sd[:], in_=sd[:])
            nc.vector.tensor_scalar(out=y_out, in0=z_psum[:],
                                    scalar1=mv[:, 0:1], scalar2=sd[:],
                                    op0=ALU.subtract, op1=ALU.mult)

        # ---------------- Stage 1: edge encoders + aggregation ----------
        for t in range(3):
            w1T, w2T, w3T = S[f"enc{t}_w1T"], S[f"enc{t}_w2T"], S[f"enc{t}_w3T"]
            b1, b2 = S[f"enc{t}_b1"], S[f"enc{t}_b2"]
            b3rep = S.get(f"enc{t}_b3rep") if m.has_b3[f"enc{t}"] else None
            for g in range(m.ngrp):
                base = g * m.slots_g
                ft = fpool.tile([m.esz, m.slots_g], F16, tag="ft", name="ft")
                nc.sync.dma_start(out=ft[:], in_=D[f"ef{t}"][:, base:base + m.slots_g])
                h1 = hpool.tile([128, m.slots_g], F16, tag="eh1", name="eh1")
                h2 = hpool.tile([128, m.slots_g], F16, tag="eh2", name="eh2")
                for sc, wd in _chunks(m.slots_g):
                    p1 = pbig.tile([128, 512], F32, tag="pb", name="pb")[:, :wd]
                    nc.tensor.matmul(p1, lhsT=w1T[:], rhs=ft[:, sc:sc + wd],
                                     start=True, stop=True)
                    nc.scalar.activation(out=h1[:, sc:sc + wd], in_=p1,
                                         func=AF.Relu, bias=b1[:], scale=1.0)
                    p2 = pbig.tile([128, 512], F32, tag="pb", name="pb")[:, :wd]
                    nc.tensor.matmul(p2, lhsT=w2T[:], rhs=h1[:, sc:sc + wd],
                                     start=True, stop=True)
                    nc.scalar.activation(out=h2[:, sc:sc + wd], in_=p2,
                                         func=AF.Relu, bias=b2[:], scale=1.0)
                sel4 = selp.tile([128, m.t_g, GSEG], F16, tag="sel", name="sel")
                nc.vector.tensor_tensor(
                    out=sel4[:],
                    in0=lidx_sb[t][:, g * m.t_g:(g + 1) * m.t_g, None]
                        .to_broadcast([128, m.t_g, GSEG]),
                    in1=iota4[:], op=ALU.is_equal)
                pg = pagg.tile([128, GSEG], F32, tag="pagg", name="pagg")
                for i in range(m.t_g):
                    zp = ptm.tile([128, 128], F32, tag="ptm", name="ptm")
                    nc.tensor.matmul(zp[:], lhsT=h2[:, i * 128:(i + 1) * 128],
                                     rhs=w3T[:], start=True, stop=True)
                    lat = latp.tile([128, 128], F16, tag="lat", name="lat")
                    layernorm_tile(zp, lat[:], b3rep)
                    nc.tensor.matmul(pg[:], lhsT=lat[:], rhs=sel4[:, i, :],
                                     start=(i == 0), stop=(i == m.t_g - 1))
                nc.vector.tensor_copy(out=agg_sb[t][:, g * GSEG:(g + 1) * GSEG],
                                      in_=pg[:])

        # ---------------- Stage 2: pre-projection of aggregation --------
        for c0, wd in _chunks(m.tpad):
            p = pbig.tile([128, 512], F32, tag="pb", name="pb")[:, :wd]
            for t in range(3):
                nc.tensor.matmul(p, lhsT=S[f"preW{t}"][:],
                                 rhs=agg_sb[t][:, c0:c0 + wd],
                                 start=(t == 0),
                                 stop=(t == 2 and not m.has_ebeta))
            if m.has_ebeta:
                nc.tensor.matmul(p, lhsT=S["betaW"][:],
                                 rhs=cnt_sb[:, c0:c0 + wd],
                                 start=False, stop=True)
            nc.scalar.activation(out=pre_sb[:, c0:c0 + wd], in_=p,
                                 func=AF.Identity, bias=S["upd_b1pre"][:], scale=1.0)

        # ---------------- Stage 3: node encoder -> x0 -------------------
        nb3 = S.get("node_b3rep") if m.has_b3["node"] else None
        for c0, wd in _chunks(m.tpad):
            p1 = pbig.tile([128, 512], F32, tag="pb", name="pb")[:, :wd]
            nc.tensor.matmul(p1, lhsT=S["node_w1T"][:],
                             rhs=nodeT_sb[:, c0:c0 + wd], start=True, stop=True)
            h1 = hpool.tile([128, 512], F16, tag="nh1", name="nh1")[:, :wd]
            nc.scalar.activation(out=h1, in_=p1, func=AF.Relu,
                                 bias=S["node_b1"][:], scale=1.0)
            p2 = pbig.tile([128, 512], F32, tag="pb", name="pb")[:, :wd]
            nc.tensor.matmul(p2, lhsT=S["node_w2T"][:], rhs=h1,
                             start=True, stop=True)
            h2 = hpool.tile([128, 512], F16, tag="nh2", name="nh2")[:, :wd]
            nc.scalar.activation(out=h2, in_=p2, func=AF.Relu,
                                 bias=S["node_b2"][:], scale=1.0)
            for i in range(wd // 128):
                zp = ptm.tile([128, 128], F32, tag="ptm", name="ptm")
                nc.tensor.matmul(zp[:], lhsT=h2[:, i * 128:(i + 1) * 128],
                                 rhs=S["node_w3T"][:], start=True, stop=True)
                y = latp.tile([128, 128], F16, tag="lat", name="lat")
                layernorm_tile(zp, y[:], nb3)
                pt = ptr.tile([128, 128], F16, tag="ptr", name="ptr")
                nc.tensor.transpose(pt[:], y[:], ident[:])
                nc.scalar.activation(out=x_sb[0][:, c0 + i * 128:c0 + (i + 1) * 128],
                                     in_=pt[:], func=AF.Identity,
                                     bias=S["node_beta"][:], scale=S["node_g"][:])

        # ---------------- Stage 4: message-passing loop ------------------
        ub3 = S.get("upd_b3rep") if m.has_b3["upd"] else None
        for s in range(10):
            xin, xout = x_sb[s % 2], x_sb[(s + 1) % 2]
            for c0, wd in _chunks(m.tpad):
                p1 = pbig.tile([128, 512], F32, tag="pb", name="pb")[:, :wd]
                nc.tensor.matmul(p1, lhsT=S["upd_w1aT"][:],
                                 rhs=xin[:, c0:c0 + wd], start=True, stop=True)
                s1 = hpool.tile([128, 512], F16, tag="mh1", name="mh1")[:, :wd]
                nc.vector.tensor_tensor(out=s1, in0=p1,
                                        in1=pre_sb[:, c0:c0 + wd], op=ALU.add)
                nc.scalar.activation(out=s1, in_=s1, func=AF.Relu)
                p2 = pbig.tile([128, 512], F32, tag="pb", name="pb")[:, :wd]
                nc.tensor.matmul(p2, lhsT=S["upd_w2T"][:], rhs=s1,
                                 start=True, stop=True)
                h2 = hpool.tile([128, 512], F16, tag="mh2", name="mh2")[:, :wd]
                nc.scalar.activation(out=h2, in_=p2, func=AF.Relu,
                                     bias=S["upd_b2"][:], scale=1.0)
                for i in range(wd // 128):
                    zp = ptm.tile([128, 128], F32, tag="ptm", name="ptm")
                    nc.tensor.matmul(zp[:], lhsT=h2[:, i * 128:(i + 1) * 128],
                                     rhs=S["upd_w3T"][:], start=True, stop=True)
                    y = latp.tile([128, 128], F16, tag="lat", name="lat")
                    layernorm_tile(zp, y[:], ub3)
                    pt = ptr.tile([128, 128], F16, tag="ptr", name="ptr")
                    nc.tensor.transpose(pt[:], y[:], ident[:])
                    nc.scalar.activation(
                        out=xout[:, c0 + i * 128:c0 + (i + 1) * 128],
                        in_=pt[:], func=AF.Identity,
                        bias=S["upd_beta"][:], scale=S["upd_g"][:])

        # ---------------- Stage 5: decoder ------------------------------
        xf = x_sb[0]
        for c0, wd in _chunks(m.tpad):
            p1 = pbig.tile([128, 512], F32, tag="pb", name="pb")[:, :wd]
            nc.tensor.matmul(p1, lhsT=S["dec_w1T"][:], rhs=xf[:, c0:c0 + wd],
                             start=True, stop=True)
            h1 = hpool.tile([128, 512], F16, tag="dh1", name="dh1")[:, :wd]
            nc.scalar.activation(out=h1, in_=p1, func=AF.Relu,
                                 bias=S["dec_b1"][:], scale=1.0)
            p2 = pbig.tile([128, 512], F32, tag="pb", name="pb")[:, :wd]
            nc.tensor.matmul(p2, lhsT=S["dec_w2T"][:], rhs=h1,
                             start=True, stop=True)
            h2 = hpool.tile([128, 512], F16, tag="dh2", name="dh2")[:, :wd]
            nc.scalar.activation(out=h2, in_=p2, func=AF.Relu,
                                 bias=S["dec_b2"][:], scale=1.0)
            p3 = pbig.tile([128, 512], F32, tag="pb", name="pb")[:m.out_sz, :wd]
            nc.tensor.matmul(p3, lhsT=S["dec_w3T"][:], rhs=h2,
                             start=True, stop=True)
            ot = outp.tile([m.out_sz, 512], F32, tag="ot", name="ot")[:, :wd]
            nc.scalar.activation(out=ot, in_=p3, func=AF.Identity,
                                 bias=S["dec_b3"][:], scale=1.0)
            nc.sync.dma_start(out=out_ap[:, c0:c0 + wd], in_=ot)

    nc.compile()
    return nc


def make_in_maps(m):
    maps = []
    for c in range(m.n_cores):
        d = dict(m.core_data[c])
        for t in range(3):
            d.pop(f"cntseg{t}", None)
        if m.has_ebeta:
            d["cnt3"] = np.stack(
                [m.core_data[c][f"cntseg{t}"] for t in range(3)])
        d.update(m.weights)
        maps.append(d)
    return maps


def kernel(**inputs):
    m = prepare_host(inputs)
    nc = build_program(m)
    maps = make_in_maps(m)
    res = run_bass_kernel_spmd(nc, maps, core_ids=list(range(m.n_cores)))
    out = np.empty((m.N, m.out_sz), np.float32)
    for c in range(m.n_cores):
        out[c * m.n_loc:(c + 1) * m.n_loc] = \
            res.results[c]["outT"][:, :m.n_loc].T
    return out


# revision 13
# speedup vs baseline: 1.3435x; 1.3435x over previous
"""Trainium2 Bass kernel for the CustomGNN message-passing network.

Strategy (node-parallel across 8 NeuronCores, no collectives needed):
  - `senders` is unused by the reference network and edge latents never
    change across the 10 MP steps, so the segment-sum aggregation is
    loop-invariant: compute it once.
  - seg_id = receiver*3 + type decomposes into 3 independent per-type
    segment sums.  Each core owns a contiguous block of N/8 nodes and
    processes exactly the edges whose receiver lands in its block, so the
    whole network (edge encoders, aggregation, node update loop, decoder)
    is embarrassingly parallel across cores.
  - On-device layout: activations are kept transposed ([feature, token]);
    every linear runs with stationary weights on the tensor engine.  The
    last linear of each MLP uses the activation tile as the stationary
    operand, producing token-major output so LayerNorm uses per-partition
    (per-token) statistics; the result returns to feature-major via DMA
    transpose.  LayerNorm gain/shift are folded into the consuming linear
    weights on the host.
  - Per-type segment sums are block matmuls: edges are sorted by receiver
    on the host and packed into groups of 64 segments with a fixed number
    of 128-edge tiles per group; a data-driven 0/1 selection matrix
    (built on gpsimd from uploaded local indices via is_equal against an
    iota) maps edge latents to segment columns accumulated in PSUM.
  - LayerNorm statistics are batched: four 128x128 token-major tiles land
    side by side in one 512-wide PSUM tile, one 3D bn_stats computes
    even/odd partial stats for all four, and cheap [128,4] vector ops
    combine them.
"""

import math
import os
import sys
import types

for _p in ("/opt/trn_rl_repo",):
    if os.path.isdir(_p) and _p not in sys.path:
        sys.path.insert(0, _p)

import numpy as np

import concourse.bass as bass
import concourse.tile as tile
from concourse import bacc, mybir
from concourse.bass_utils import run_bass_kernel_spmd

F32 = mybir.dt.float32
F16 = mybir.dt.float16
AF = mybir.ActivationFunctionType
ALU = mybir.AluOpType

N_CORES = 8
GSEG = 64  # segments per aggregation group
EPS = 1e-5


def _np(a, dt=np.float32):
    return np.asarray(a).astype(dt)


def _mlp_arrays(mlp):
    Ws = [_np(W) for W in mlp["Ws"]]
    bs = [_np(b) for b in mlp["bs"]]
    g = _np(mlp["g"]) if mlp.get("g") is not None else None
    beta = _np(mlp["beta"]) if mlp.get("beta") is not None else None
    return Ws, bs, g, beta


def _col(v):
    return np.ascontiguousarray(np.asarray(v).reshape(-1, 1).astype(np.float32))


class Meta:
    pass


def prepare_host(inputs, n_cores=N_CORES):
    """Shard + sort edges, pack per-core device buffers, prep weights."""
    m = Meta()
    node_feats = _np(inputs["node_feats"])
    feats_by_type = [_np(inputs["body_feats"]), _np(inputs["cable_feats"]),
                     _np(inputs["con_feats"])]
    recv = np.asarray(inputs["receivers"]).astype(np.int64)
    N = node_feats.shape[0]
    assert N % n_cores == 0
    n_loc = N // n_cores
    tpad = ((n_loc + 127) // 128) * 128
    ngrp = tpad // GSEG
    m.N, m.n_loc, m.tpad, m.ngrp = N, n_loc, tpad, ngrp
    m.node_sz = node_feats.shape[1]
    m.esz = feats_by_type[0].shape[1]
    m.n_cores = n_cores

    ne = [f.shape[0] for f in feats_by_type]
    off = np.cumsum([0] + ne)
    per_ct = [[None] * 3 for _ in range(n_cores)]
    maxcnt = 1
    for t in range(3):
        r_t = recv[off[t]:off[t + 1]]
        order = np.argsort(r_t, kind="stable")
        r_sorted = r_t[order]
        bounds = np.searchsorted(r_sorted, np.arange(0, N + 1, n_loc))
        for c in range(n_cores):
            sl = slice(bounds[c], bounds[c + 1])
            lr = r_sorted[sl] - c * n_loc
            idx = order[sl]
            grp = lr // GSEG
            cnt = np.bincount(grp, minlength=ngrp)
            maxcnt = max(maxcnt, int(cnt.max()) if len(cnt) else 1)
            per_ct[c][t] = (lr, idx, grp, cnt)
    t_g = (maxcnt + 127) // 128
    slots_g = t_g * 128
    e_slots = ngrp * slots_g
    m.t_g, m.slots_g, m.e_slots, m.nt = t_g, slots_g, e_slots, ngrp * t_g

    m.core_data = []
    for c in range(n_cores):
        d = {}
        for t in range(3):
            lr, idx, grp, cnt = per_ct[c][t]
            gstart = np.cumsum(cnt) - cnt
            rank = np.arange(len(lr)) - gstart[grp]
            slot = grp * slots_g + rank
            fT = np.zeros((m.esz, e_slots), np.float16)
            fT[:, slot] = feats_by_type[t][idx].T
            lidxb = np.full(e_slots, 127.0, np.float16)
            lidxb[slot] = (lr % GSEG).astype(np.float16)
            d[f"ef{t}"] = fT
            d[f"lidx{t}"] = np.ascontiguousarray(lidxb.reshape(m.nt, 128).T)
            d[f"cntseg{t}"] = np.bincount(lr, minlength=tpad).astype(np.float16)
        nT = np.zeros((m.node_sz, tpad), np.float16)
        nT[:, :n_loc] = node_feats[c * n_loc:(c + 1) * n_loc].T
        d["nodeT"] = nT
        m.core_data.append(d)

    # ---- weights ----
    w = {}
    enc_keys = ["body_enc", "cable_enc", "con_enc"]
    uWs, ubs, ug, ubeta = _mlp_arrays(inputs["node_upd"])
    nWs, nbs, ng, nbeta = _mlp_arrays(inputs["node_enc"])
    L = uWs[-1].shape[0]
    m.L = L
    m.has_b3 = {}
    m.beta_vecs = []
    for t in range(3):
        Ws, bs, g, beta = _mlp_arrays(inputs[enc_keys[t]])
        w[f"enc{t}_w1T"] = Ws[0].T.astype(np.float16)
        w[f"enc{t}_w2T"] = Ws[1].T.astype(np.float16)
        w[f"enc{t}_w3T"] = Ws[2].T.astype(np.float16)
        w[f"enc{t}_b1"] = _col(bs[0])
        w[f"enc{t}_b2"] = _col(bs[1])
        m.has_b3[f"enc{t}"] = bool(np.any(bs[2]))
        if m.has_b3[f"enc{t}"]:
            w[f"enc{t}_b3rep"] = np.tile(bs[2].astype(np.float16), (128, 4))
        # fold edge-LN g into the pre-projection, beta via counts
        W1b_t = uWs[0][:, L * (t + 1):L * (t + 2)]
        w[f"preW{t}"] = (W1b_t.T * g[:, None]).astype(np.float16)
        m.beta_vecs.append(W1b_t @ beta)
    m.has_ebeta = bool(any(np.any(v) for v in m.beta_vecs))
    if m.has_ebeta:
        w["betaW"] = np.stack(m.beta_vecs).astype(np.float16)  # [3, 128]

    w["node_w1T"] = nWs[0].T.astype(np.float16)
    w["node_w2T"] = nWs[1].T.astype(np.float16)
    w["node_w3T"] = nWs[2].T.astype(np.float16)
    w["node_b1"] = _col(nbs[0])
    w["node_b2"] = _col(nbs[1])
    m.has_b3["node"] = bool(np.any(nbs[2]))
    if m.has_b3["node"]:
        w["node_b3rep"] = np.tile(nbs[2].astype(np.float16), (128, 4))

    # node-update MLP; L1 split into x-part (with LN folds) and agg-part
    W1a = uWs[0][:, :L]
    w["upd_w1gTn"] = (W1a.T * ng[:, None]).astype(np.float16)  # step 0
    w["upd_w1gTu"] = (W1a.T * ug[:, None]).astype(np.float16)  # steps 1-9
    w["upd_w2T"] = uWs[1].T.astype(np.float16)
    w["upd_w3T"] = uWs[2].T.astype(np.float16)
    w["b1pre_n"] = _col(ubs[0] + W1a @ nbeta)
    w["b1pre_u"] = _col(ubs[0] + W1a @ ubeta)
    w["upd_b2"] = _col(ubs[1])
    m.has_b3["upd"] = bool(np.any(ubs[2]))
    if m.has_b3["upd"]:
        w["upd_b3rep"] = np.tile(ubs[2].astype(np.float16), (128, 4))

    dWs, dbs, _, _ = _mlp_arrays(inputs["dec"])
    w["dec_w1T"] = (dWs[0].T * ug[:, None]).astype(np.float16)
    w["dec_w2T"] = dWs[1].T.astype(np.float16)
    w["dec_w3T"] = dWs[2].T.astype(np.float16)
    w["dec_b1"] = _col(dbs[0] + dWs[0] @ ubeta)
    w["dec_b2"] = _col(dbs[1])
    w["dec_b3"] = _col(dbs[2])
    m.out_sz = dWs[2].shape[0]

    w["ident"] = np.eye(128, dtype=np.float16)
    w["iota4"] = np.tile(np.arange(GSEG, dtype=np.float16), (128, t_g, 1))
    m.weights = w
    m.two_pre = bool(np.any(w["b1pre_n"] != w["b1pre_u"])
                     or np.any(w["upd_w1gTn"] != w["upd_w1gTu"]))
    return m


def _chunks(total, step=512):
    out = []
    c = 0
    while c < total:
        out.append((c, min(step, total - c)))
        c += step
    return out


def build_program(m):
    nc = bacc.Bacc("TRN2", target_bir_lowering=False, debug=False)
    D = {}

    def din(name, shape, dt):
        D[name] = nc.dram_tensor(name, list(shape), dt, kind="ExternalInput").ap()

    for t in range(3):
        din(f"ef{t}", (m.esz, m.e_slots), F16)
        din(f"lidx{t}", (128, m.nt), F16)
    if m.has_ebeta:
        din("cnt3", (3, m.tpad), F16)
    din("nodeT", (m.node_sz, m.tpad), F16)
    for k, v in m.weights.items():
        din(k, v.shape, F16 if v.dtype == np.float16 else F32)
    out_ap = nc.dram_tensor("outT", [m.out_sz, m.tpad], F32,
                            kind="ExternalOutput").ap()

    from contextlib import ExitStack
    with tile.TileContext(nc) as tc, ExitStack() as ctx:
        sing = ctx.enter_context(tc.tile_pool(name="sing", bufs=1))
        big = ctx.enter_context(tc.tile_pool(name="big", bufs=1))
        fpool = ctx.enter_context(tc.tile_pool(name="fpool", bufs=3))
        hpool = ctx.enter_context(tc.tile_pool(name="hpool", bufs=3))
        latp = ctx.enter_context(tc.tile_pool(name="latp", bufs=3))
        selp = ctx.enter_context(tc.tile_pool(name="selp", bufs=3))
        stp = ctx.enter_context(tc.tile_pool(name="stp", bufs=6))
        outp = ctx.enter_context(tc.tile_pool(name="outp", bufs=3))
        pbig = ctx.enter_context(tc.tile_pool(name="pbig", bufs=3, space="PSUM"))
        pzp = ctx.enter_context(tc.tile_pool(name="pzp", bufs=3, space="PSUM"))
        pagg = ctx.enter_context(tc.tile_pool(name="pagg", bufs=2, space="PSUM"))

        # resident SBUF tensors
        S = {}
        for k, v in m.weights.items():
            dt = F16 if v.dtype == np.float16 else F32
            S[k] = sing.tile(list(v.shape), dt, name=k, tag=k)
            nc.sync.dma_start(out=S[k][:], in_=D[k][:])
        lidx_sb = []
        for t in range(3):
            lt = sing.tile([128, m.nt], F16, name=f"lidx{t}", tag=f"lidx{t}")
            nc.sync.dma_start(out=lt[:], in_=D[f"lidx{t}"][:])
            lidx_sb.append(lt)
        if m.has_ebeta:
            cnt_sb = sing.tile([3, m.tpad], F16, name="cnt_sb", tag="cnt_sb")
            nc.sync.dma_start(out=cnt_sb[:], in_=D["cnt3"][:])
        nodeT_sb = sing.tile([m.node_sz, m.tpad], F16, name="nodeT_sb",
                             tag="nodeT_sb")
        nc.sync.dma_start(out=nodeT_sb[:], in_=D["nodeT"][:])
        eps_sb = sing.tile([128, 1], F32, name="eps_sb", tag="eps_sb")
        nc.vector.memset(eps_sb[:], EPS)

        agg_sb = [big.tile([128, m.tpad], F16, tag=f"agg{t}", name=f"agg{t}")
                  for t in range(3)]
        x_sb = [big.tile([128, m.tpad], F16, tag=f"x{i}", name=f"x{i}")
                for i in range(2)]
        pre0 = big.tile([128, m.tpad], F16, tag="pre0", name="pre0")
        preK = (big.tile([128, m.tpad], F16, tag="preK", name="preK")
                if m.two_pre else pre0)

        ident = S["ident"]
        iota4 = S["iota4"]

        def layernorm_batch(pz, nt_sub, y_out, b3rep):
            """nt_sub token-major [128,128] psum slabs -> normalized SBUF.

            pz: [128, nt_sub*128] psum (token-major subtiles side by side)
            y_out: [128, nt_sub*128] fp16 SBUF destination
            """
            if b3rep is not None:
                nc.vector.tensor_tensor(out=pz[:], in0=pz[:],
                                        in1=b3rep[:, :nt_sub * 128],
                                        op=ALU.add)
            st = stp.tile([128, 4, 6], F32, tag="st", name="st")[:, :nt_sub, :]
            for i in range(nt_sub):
                nc.vector.bn_stats(out=st[:, i, :],
                                   in_=pz[:, i * 128:(i + 1) * 128])
            s1, s2 = st[:, :, 1], st[:, :, 2]
            s4, s5 = st[:, :, 4], st[:, :, 5]
            t2 = stp.tile([128, 4], F32, tag="t2", name="t2")[:, :nt_sub]
            nc.vector.tensor_tensor(out=t2, in0=s2, in1=s5, op=ALU.add)
            dm = stp.tile([128, 4], F32, tag="dm", name="dm")[:, :nt_sub]
            nc.vector.tensor_tensor(out=dm, in0=s1, in1=s4, op=ALU.subtract)
            d2q = stp.tile([128, 4], F32, tag="d2q", name="d2q")[:, :nt_sub]
            nc.vector.scalar_tensor_tensor(out=d2q, in0=dm, scalar=0.25,
                                           in1=dm, op0=ALU.mult, op1=ALU.mult)
            var = stp.tile([128, 4], F32, tag="var", name="var")[:, :nt_sub]
            nc.vector.scalar_tensor_tensor(out=var, in0=t2, scalar=1.0 / 128,
                                           in1=d2q, op0=ALU.mult, op1=ALU.add)
            m2x = stp.tile([128, 4], F32, tag="m2x", name="m2x")[:, :nt_sub]
            nc.vector.tensor_tensor(out=m2x, in0=s1, in1=s4, op=ALU.add)
            sd = stp.tile([128, 4], F32, tag="sd", name="sd")[:, :nt_sub]
            nc.scalar.activation(out=sd, in_=var, func=AF.Sqrt,
                                 bias=eps_sb[:], scale=1.0)
            rstd = stp.tile([128, 4], F32, tag="rstd", name="rstd")[:, :nt_sub]
            nc.vector.reciprocal(out=rstd, in_=sd)
            nmr = stp.tile([128, 4], F32, tag="nmr", name="nmr")[:, :nt_sub]
            nc.vector.scalar_tensor_tensor(out=nmr, in0=m2x, scalar=-0.5,
                                           in1=rstd, op0=ALU.mult, op1=ALU.mult)
            for i in range(nt_sub):
                nc.any.tensor_scalar(
                    out=y_out[:, i * 128:(i + 1) * 128],
                    in0=pz[:, i * 128:(i + 1) * 128],
                    scalar1=rstd[:, i:i + 1], scalar2=nmr[:, i:i + 1],
                    op0=ALU.mult, op1=ALU.add)

        # ---------------- Stage 1: edge encoders + aggregation ----------
        for t in range(3):
            w1T, w2T, w3T = S[f"enc{t}_w1T"], S[f"enc{t}_w2T"], S[f"enc{t}_w3T"]
            b1, b2 = S[f"enc{t}_b1"], S[f"enc{t}_b2"]
            b3rep = S.get(f"enc{t}_b3rep") if m.has_b3[f"enc{t}"] else None
            for g in range(m.ngrp):
                base = g * m.slots_g
                ft = fpool.tile([m.esz, m.slots_g], F16, tag="ft", name="ft")
                nc.sync.dma_start(out=ft[:],
                                  in_=D[f"ef{t}"][:, base:base + m.slots_g])
                h2 = hpool.tile([128, m.slots_g], F16, tag="eh2", name="eh2")
                for sc, wd in _chunks(m.slots_g):
                    p1 = pbig.tile([128, 512], F32, tag="pb", name="pb")[:, :wd]
                    nc.tensor.matmul(p1, lhsT=w1T[:], rhs=ft[:, sc:sc + wd],
                                     start=True, stop=True)
                    h1 = hpool.tile([128, 512], F16, tag="eh1",
                                    name="eh1")[:, :wd]
                    nc.scalar.activation(out=h1, in_=p1, func=AF.Relu,
                                         bias=b1[:], scale=1.0)
                    p2 = pbig.tile([128, 512], F32, tag="pb", name="pb")[:, :wd]
                    nc.tensor.matmul(p2, lhsT=w2T[:], rhs=h1,
                                     start=True, stop=True)
                    nc.scalar.activation(out=h2[:, sc:sc + wd], in_=p2,
                                         func=AF.Relu, bias=b2[:], scale=1.0)
                sel4 = selp.tile([128, m.t_g, GSEG], F16, tag="sel", name="sel")
                nc.vector.tensor_tensor(
                    out=sel4[:],
                    in0=lidx_sb[t][:, g * m.t_g:(g + 1) * m.t_g, None]
                        .to_broadcast([128, m.t_g, GSEG]),
                    in1=iota4[:], op=ALU.is_equal)
                pg = pagg.tile([128, GSEG], F32, tag="pagg", name="pagg")
                for tb in range(0, m.t_g, 4):
                    nt_sub = min(4, m.t_g - tb)
                    sw = nt_sub * 128
                    pz = pzp.tile([128, 512], F32, tag="pz", name="pz")[:, :sw]
                    for i in range(nt_sub):
                        j = tb + i
                        nc.tensor.matmul(pz[:, i * 128:(i + 1) * 128],
                                         lhsT=h2[:, j * 128:(j + 1) * 128],
                                         rhs=w3T[:], start=True, stop=True)
                    lat = latp.tile([128, 512], F16, tag="lat",
                                    name="lat")[:, :sw]
                    layernorm_batch(pz, nt_sub, lat, b3rep)
                    for i in range(nt_sub):
                        j = tb + i
                        nc.tensor.matmul(pg[:],
                                         lhsT=lat[:, i * 128:(i + 1) * 128],
                                         rhs=sel4[:, j, :],
                                         start=(j == 0),
                                         stop=(j == m.t_g - 1))
                nc.any.tensor_copy(out=agg_sb[t][:, g * GSEG:(g + 1) * GSEG],
                                   in_=pg[:])

        # ---------------- Stage 2: pre-projection of aggregation --------
        for c0, wd in _chunks(m.tpad):
            p = pbig.tile([128, 512], F32, tag="pb", name="pb")[:, :wd]
            for t in range(3):
                nc.tensor.matmul(p, lhsT=S[f"preW{t}"][:],
                                 rhs=agg_sb[t][:, c0:c0 + wd],
                                 start=(t == 0),
                                 stop=(t == 2 and not m.has_ebeta))
            if m.has_ebeta:
                nc.tensor.matmul(p, lhsT=S["betaW"][:],
                                 rhs=cnt_sb[:, c0:c0 + wd],
                                 start=False, stop=True)
            nc.scalar.activation(out=pre0[:, c0:c0 + wd], in_=p,
                                 func=AF.Identity, bias=S["b1pre_n"][:],
                                 scale=1.0)
            if m.two_pre:
                nc.scalar.activation(out=preK[:, c0:c0 + wd], in_=p,
                                     func=AF.Identity, bias=S["b1pre_u"][:],
                                     scale=1.0)

        # ---------------- Stage 3: node encoder -> x0 (raw-normalized) --
        nb3 = S.get("node_b3rep") if m.has_b3["node"] else None
        for c0, wd in _chunks(m.tpad):
            p1 = pbig.tile([128, 512], F32, tag="pb", name="pb")[:, :wd]
            nc.tensor.matmul(p1, lhsT=S["node_w1T"][:],
                             rhs=nodeT_sb[:, c0:c0 + wd], start=True, stop=True)
            h1 = hpool.tile([128, 512], F16, tag="nh1", name="nh1")[:, :wd]
            nc.scalar.activation(out=h1, in_=p1, func=AF.Relu,
                                 bias=S["node_b1"][:], scale=1.0)
            p2 = pbig.tile([128, 512], F32, tag="pb", name="pb")[:, :wd]
            nc.tensor.matmul(p2, lhsT=S["node_w2T"][:], rhs=h1,
                             start=True, stop=True)
            h2 = hpool.tile([128, 512], F16, tag="nh2", name="nh2")[:, :wd]
            nc.scalar.activation(out=h2, in_=p2, func=AF.Relu,
                                 bias=S["node_b2"][:], scale=1.0)
            nt_sub = wd // 128
            pz = pzp.tile([128, 512], F32, tag="pz", name="pz")[:, :wd]
            for i in range(nt_sub):
                nc.tensor.matmul(pz[:, i * 128:(i + 1) * 128],
                                 lhsT=h2[:, i * 128:(i + 1) * 128],
                                 rhs=S["node_w3T"][:], start=True, stop=True)
            y4 = latp.tile([128, 512], F16, tag="y4", name="y4")[:, :wd]
            layernorm_batch(pz, nt_sub, y4, nb3)
            for i in range(nt_sub):
                nc.sync.dma_start(
                    out=x_sb[0][:, c0 + i * 128:c0 + (i + 1) * 128],
                    in_=y4[:, i * 128:(i + 1) * 128], transpose=True)

        # ---------------- Stage 4: message-passing loop ------------------
        ub3 = S.get("upd_b3rep") if m.has_b3["upd"] else None
        for s in range(10):
            xin, xout = x_sb[s % 2], x_sb[(s + 1) % 2]
            w1gT = S["upd_w1gTn"] if s == 0 else S["upd_w1gTu"]
            pre_x = pre0 if s == 0 else preK
            for c0, wd in _chunks(m.tpad):
                p1 = pbig.tile([128, 512], F32, tag="pb", name="pb")[:, :wd]
                nc.tensor.matmul(p1, lhsT=ident[:], rhs=pre_x[:, c0:c0 + wd],
                                 start=True, stop=False)
                nc.tensor.matmul(p1, lhsT=w1gT[:], rhs=xin[:, c0:c0 + wd],
                                 start=False, stop=True)
                h1 = hpool.tile([128, 512], F16, tag="mh1", name="mh1")[:, :wd]
                nc.scalar.activation(out=h1, in_=p1, func=AF.Relu)
                p2 = pbig.tile([128, 512], F32, tag="pb", name="pb")[:, :wd]
                nc.tensor.matmul(p2, lhsT=S["upd_w2T"][:], rhs=h1,
                                 start=True, stop=True)
                h2 = hpool.tile([128, 512], F16, tag="mh2", name="mh2")[:, :wd]
                nc.scalar.activation(out=h2, in_=p2, func=AF.Relu,
                                     bias=S["upd_b2"][:], scale=1.0)
                nt_sub = wd // 128
                pz = pzp.tile([128, 512], F32, tag="pz", name="pz")[:, :wd]
                for i in range(nt_sub):
                    nc.tensor.matmul(pz[:, i * 128:(i + 1) * 128],
                                     lhsT=h2[:, i * 128:(i + 1) * 128],
                                     rhs=S["upd_w3T"][:], start=True, stop=True)
                y4 = latp.tile([128, 512], F16, tag="y4", name="y4")[:, :wd]
                layernorm_batch(pz, nt_sub, y4, ub3)
                for i in range(nt_sub):
                    nc.sync.dma_start(
                        out=xout[:, c0 + i * 128:c0 + (i + 1) * 128],
                        in_=y4[:, i * 128:(i + 1) * 128], transpose=True)

        # ---------------- Stage 5: decoder ------------------------------
        xf = x_sb[0]
        for c0, wd in _chunks(m.tpad):
            p1 = pbig.tile([128, 512], F32, tag="pb", name="pb")[:, :wd]
            nc.tensor.matmul(p1, lhsT=S["dec_w1T"][:], rhs=xf[:, c0:c0 + wd],
                             start=True, stop=True)
            h1 = hpool.tile([128, 512], F16, tag="dh1", name="dh1")[:, :wd]
            nc.scalar.activation(out=h1, in_=p1, func=AF.Relu,
                                 bias=S["dec_b1"][:], scale=1.0)
            p2 = pbig.tile([128, 512], F32, tag="pb", name="pb")[:, :wd]
            nc.tensor.matmul(p2, lhsT=S["dec_w2T"][:], rhs=h1,
                             start=True, stop=True)
            h2 = hpool.tile([128, 512], F16, tag="dh2", name="dh2")[:, :wd]
            nc.scalar.activation(out=h2, in_=p2, func=AF.Relu,
                                 bias=S["dec_b2"][:], scale=1.0)
            p3 = pbig.tile([128, 512], F32, tag="pb",
                           name="pb")[:m.out_sz, :wd]
            nc.tensor.matmul(p3, lhsT=S["dec_w3T"][:], rhs=h2,
                             start=True, stop=True)
            ot = outp.tile([m.out_sz, 512], F32, tag="ot", name="ot")[:, :wd]
            nc.scalar.activation(out=ot, in_=p3, func=AF.Identity,
                                 bias=S["dec_b3"][:], scale=1.0)
            nc.sync.dma_start(out=out_ap[:, c0:c0 + wd], in_=ot)

    nc.compile()
    return nc


def make_in_maps(m):
    maps = []
    for c in range(m.n_cores):
        d = dict(m.core_data[c])
        for t in range(3):
            d.pop(f"cntseg{t}", None)
        if m.has_ebeta:
            d["cnt3"] = np.stack(
                [m.core_data[c][f"cntseg{t}"] for t in range(3)])
        d.update(m.weights)
        maps.append(d)
    return maps


def kernel(**inputs):
    m = prepare_host(inputs)
    nc = build_program(m)
    maps = make_in_maps(m)
    res = run_bass_kernel_spmd(nc, maps, core_ids=list(range(m.n_cores)))
    out = np.empty((m.N, m.out_sz), np.float32)
    for c in range(m.n_cores):
        out[c * m.n_loc:(c + 1) * m.n_loc] = \
            res.results[c]["outT"][:, :m.n_loc].T
    return out


# revision 17
# speedup vs baseline: 1.6798x; 1.2504x over previous
"""Trainium2 Bass kernel for the CustomGNN message-passing network.

Strategy (node-parallel across 8 NeuronCores, no collectives needed):
  - `senders` is unused by the reference network and edge latents never
    change across the 10 MP steps, so the segment-sum aggregation is
    loop-invariant: compute it once.
  - seg_id = receiver*3 + type decomposes into 3 independent per-type
    segment sums.  Each core owns a contiguous block of N/8 nodes and
    processes exactly the edges whose receiver lands in its block, so the
    whole network (edge encoders, aggregation, node update loop, decoder)
    is embarrassingly parallel across cores.
  - On-device layout: activations are kept transposed ([feature, token]);
    every linear runs with stationary weights on the tensor engine.  The
    last linear of each MLP uses the activation tile as the stationary
    operand, producing token-major output so LayerNorm uses per-partition
    (per-token) statistics; the result returns to feature-major via DMA
    transpose.  LayerNorm gain/shift are folded into the consuming linear
    weights on the host.
  - Per-type segment sums are block matmuls: edges are sorted by receiver
    on the host and packed into groups of 64 segments with a fixed number
    of 128-edge tiles per group; a data-driven 0/1 selection matrix
    (built on gpsimd from uploaded local indices via is_equal against an
    iota) maps edge latents to segment columns accumulated in PSUM.
  - LayerNorm statistics are batched: four 128x128 token-major tiles land
    side by side in one 512-wide PSUM tile, one 3D bn_stats computes
    even/odd partial stats for all four, and cheap [128,4] vector ops
    combine them.
"""

import math
import os
import sys
import types

for _p in ("/opt/trn_rl_repo",):
    if os.path.isdir(_p) and _p not in sys.path:
        sys.path.insert(0, _p)

import numpy as np

import concourse.bass as bass
import concourse.tile as tile
from concourse import bacc, mybir
from concourse.bass_utils import run_bass_kernel_spmd

F32 = mybir.dt.float32
F16 = mybir.dt.float16
AF = mybir.ActivationFunctionType
ALU = mybir.AluOpType

N_CORES = 8
GSEG = 64  # segments per aggregation group
EPS = 1e-5


def _np(a, dt=np.float32):
    return np.asarray(a).astype(dt)


def _mlp_arrays(mlp):
    Ws = [_np(W) for W in mlp["Ws"]]
    bs = [_np(b) for b in mlp["bs"]]
    g = _np(mlp["g"]) if mlp.get("g") is not None else None
    beta = _np(mlp["beta"]) if mlp.get("beta") is not None else None
    return Ws, bs, g, beta


def _col(v):
    return np.ascontiguousarray(np.asarray(v).reshape(-1, 1).astype(np.float32))


class Meta:
    pass


def prepare_host(inputs, n_cores=N_CORES):
    """Shard + sort edges, pack per-core device buffers, prep weights."""
    m = Meta()
    node_feats = _np(inputs["node_feats"])
    feats_by_type = [_np(inputs["body_feats"]), _np(inputs["cable_feats"]),
                     _np(inputs["con_feats"])]
    recv = np.asarray(inputs["receivers"]).astype(np.int64)
    N = node_feats.shape[0]
    assert N % n_cores == 0
    n_loc = N // n_cores
    tpad = ((n_loc + 127) // 128) * 128
    ngrp = tpad // GSEG
    m.N, m.n_loc, m.tpad, m.ngrp = N, n_loc, tpad, ngrp
    m.node_sz = node_feats.shape[1]
    m.esz = feats_by_type[0].shape[1]
    m.n_cores = n_cores

    ne = [f.shape[0] for f in feats_by_type]
    off = np.cumsum([0] + ne)
    per_ct = [[None] * 3 for _ in range(n_cores)]
    maxcnt = 1
    for t in range(3):
        r_t = recv[off[t]:off[t + 1]]
        order = np.argsort(r_t, kind="stable")
        r_sorted = r_t[order]
        bounds = np.searchsorted(r_sorted, np.arange(0, N + 1, n_loc))
        for c in range(n_cores):
            sl = slice(bounds[c], bounds[c + 1])
            lr = r_sorted[sl] - c * n_loc
            idx = order[sl]
            grp = lr // GSEG
            cnt = np.bincount(grp, minlength=ngrp)
            maxcnt = max(maxcnt, int(cnt.max()) if len(cnt) else 1)
            per_ct[c][t] = (lr, idx, grp, cnt)
    t_g = (maxcnt + 127) // 128
    slots_g = t_g * 128
    e_slots = ngrp * slots_g
    m.t_g, m.slots_g, m.e_slots, m.nt = t_g, slots_g, e_slots, ngrp * t_g

    m.core_data = []
    for c in range(n_cores):
        d = {}
        for t in range(3):
            lr, idx, grp, cnt = per_ct[c][t]
            gstart = np.cumsum(cnt) - cnt
            rank = np.arange(len(lr)) - gstart[grp]
            slot = grp * slots_g + rank
            fT = np.zeros((m.esz, e_slots), np.float16)
            fT[:, slot] = feats_by_type[t][idx].T
            lidxb = np.full(e_slots, 127.0, np.float16)
            lidxb[slot] = (lr % GSEG).astype(np.float16)
            d[f"ef{t}"] = fT
            d[f"lidx{t}"] = np.ascontiguousarray(lidxb.reshape(m.nt, 128).T)
            d[f"cntseg{t}"] = np.bincount(lr, minlength=tpad).astype(np.float16)
        nT = np.zeros((m.node_sz, tpad), np.float16)
        nT[:, :n_loc] = node_feats[c * n_loc:(c + 1) * n_loc].T
        d["nodeT"] = nT
        m.core_data.append(d)

    # ---- weights ----
    w = {}
    enc_keys = ["body_enc", "cable_enc", "con_enc"]
    uWs, ubs, ug, ubeta = _mlp_arrays(inputs["node_upd"])
    nWs, nbs, ng, nbeta = _mlp_arrays(inputs["node_enc"])
    L = uWs[-1].shape[0]
    m.L = L
    m.has_b3 = {}
    m.beta_vecs = []
    for t in range(3):
        Ws, bs, g, beta = _mlp_arrays(inputs[enc_keys[t]])
        w[f"enc{t}_w1T"] = Ws[0].T.astype(np.float16)
        w[f"enc{t}_w2T"] = Ws[1].T.astype(np.float16)
        w[f"enc{t}_w3T"] = Ws[2].T.astype(np.float16)
        w[f"enc{t}_b1"] = _col(bs[0])
        w[f"enc{t}_b2"] = _col(bs[1])
        m.has_b3[f"enc{t}"] = bool(np.any(bs[2]))
        if m.has_b3[f"enc{t}"]:
            w[f"enc{t}_b3rep"] = np.tile(bs[2].astype(np.float16), (128, 4))
        # fold edge-LN g into the pre-projection, beta via counts
        W1b_t = uWs[0][:, L * (t + 1):L * (t + 2)]
        w[f"preW{t}"] = (W1b_t.T * g[:, None]).astype(np.float16)
        m.beta_vecs.append(W1b_t @ beta)
    m.has_ebeta = bool(any(np.any(v) for v in m.beta_vecs))
    if m.has_ebeta:
        w["betaW"] = np.stack(m.beta_vecs).astype(np.float16)  # [3, 128]

    w["node_w1T"] = nWs[0].T.astype(np.float16)
    w["node_w2T"] = nWs[1].T.astype(np.float16)
    w["node_w3T"] = nWs[2].T.astype(np.float16)
    w["node_b1"] = _col(nbs[0])
    w["node_b2"] = _col(nbs[1])
    m.has_b3["node"] = bool(np.any(nbs[2]))
    if m.has_b3["node"]:
        w["node_b3rep"] = np.tile(nbs[2].astype(np.float16), (128, 4))

    # node-update MLP; L1 split into x-part (with LN folds) and agg-part
    W1a = uWs[0][:, :L]
    w["upd_w1gTn"] = (W1a.T * ng[:, None]).astype(np.float16)  # step 0
    w["upd_w1gTu"] = (W1a.T * ug[:, None]).astype(np.float16)  # steps 1-9
    w["upd_w2T"] = uWs[1].T.astype(np.float16)
    w["upd_w3T"] = uWs[2].T.astype(np.float16)
    w["b1pre_n"] = _col(ubs[0] + W1a @ nbeta)
    w["b1pre_u"] = _col(ubs[0] + W1a @ ubeta)
    w["upd_b2"] = _col(ubs[1])
    m.has_b3["upd"] = bool(np.any(ubs[2]))
    if m.has_b3["upd"]:
        w["upd_b3rep"] = np.tile(ubs[2].astype(np.float16), (128, 4))

    dWs, dbs, _, _ = _mlp_arrays(inputs["dec"])
    w["dec_w1T"] = (dWs[0].T * ug[:, None]).astype(np.float16)
    w["dec_w2T"] = dWs[1].T.astype(np.float16)
    w["dec_w3T"] = dWs[2].T.astype(np.float16)
    w["dec_b1"] = _col(dbs[0] + dWs[0] @ ubeta)
    w["dec_b2"] = _col(dbs[1])
    w["dec_b3"] = _col(dbs[2])
    m.out_sz = dWs[2].shape[0]

    w["ident"] = np.eye(128, dtype=np.float16)
    w["iota4"] = np.tile(np.arange(GSEG, dtype=np.float16), (128, t_g, 1))
    m.weights = w
    m.two_pre = bool(np.any(w["b1pre_n"] != w["b1pre_u"])
                     or np.any(w["upd_w1gTn"] != w["upd_w1gTu"]))
    return m


def _chunks(total, step=512):
    out = []
    c = 0
    while c < total:
        out.append((c, min(step, total - c)))
        c += step
    return out


def build_program(m):
    nc = bacc.Bacc("TRN2", target_bir_lowering=False, debug=False)
    D = {}

    def din(name, shape, dt):
        D[name] = nc.dram_tensor(name, list(shape), dt, kind="ExternalInput").ap()

    for t in range(3):
        din(f"ef{t}", (m.esz, m.e_slots), F16)
        din(f"lidx{t}", (128, m.nt), F16)
    if m.has_ebeta:
        din("cnt3", (3, m.tpad), F16)
    din("nodeT", (m.node_sz, m.tpad), F16)
    for k, v in m.weights.items():
        din(k, v.shape, F16 if v.dtype == np.float16 else F32)
    out_ap = nc.dram_tensor("outT", [m.out_sz, m.tpad], F32,
                            kind="ExternalOutput").ap()

    from contextlib import ExitStack
    with tile.TileContext(nc) as tc, ExitStack() as ctx:
        sing = ctx.enter_context(tc.tile_pool(name="sing", bufs=1))
        big = ctx.enter_context(tc.tile_pool(name="big", bufs=1))
        fpool = ctx.enter_context(tc.tile_pool(name="fpool", bufs=3))
        hpool = ctx.enter_context(tc.tile_pool(name="hpool", bufs=3))
        latp = ctx.enter_context(tc.tile_pool(name="latp", bufs=3))
        selp = ctx.enter_context(tc.tile_pool(name="selp", bufs=3))
        stp = ctx.enter_context(tc.tile_pool(name="stp", bufs=6))
        zpool = ctx.enter_context(tc.tile_pool(name="zpool", bufs=3))
        outp = ctx.enter_context(tc.tile_pool(name="outp", bufs=3))
        pbig = ctx.enter_context(tc.tile_pool(name="pbig", bufs=2, space="PSUM"))
        pzp = ctx.enter_context(tc.tile_pool(name="pzp", bufs=2, space="PSUM"))
        pagg = ctx.enter_context(tc.tile_pool(name="pagg", bufs=2, space="PSUM"))
        ptrp = ctx.enter_context(tc.tile_pool(name="ptrp", bufs=2, space="PSUM"))

        # resident SBUF tensors
        S = {}
        for k, v in m.weights.items():
            dt = F16 if v.dtype == np.float16 else F32
            S[k] = sing.tile(list(v.shape), dt, name=k, tag=k)
            nc.sync.dma_start(out=S[k][:], in_=D[k][:])
        lidx_sb = []
        for t in range(3):
            lt = sing.tile([128, m.nt], F16, name=f"lidx{t}", tag=f"lidx{t}")
            nc.sync.dma_start(out=lt[:], in_=D[f"lidx{t}"][:])
            lidx_sb.append(lt)
        if m.has_ebeta:
            cnt_sb = sing.tile([3, m.tpad], F16, name="cnt_sb", tag="cnt_sb")
            nc.sync.dma_start(out=cnt_sb[:], in_=D["cnt3"][:])
        nodeT_sb = sing.tile([m.node_sz, m.tpad], F16, name="nodeT_sb",
                             tag="nodeT_sb")
        nc.sync.dma_start(out=nodeT_sb[:], in_=D["nodeT"][:])
        eps_sb = sing.tile([128, 1], F32, name="eps_sb", tag="eps_sb")
        nc.vector.memset(eps_sb[:], EPS)

        agg_sb = [big.tile([128, m.tpad], F16, tag=f"agg{t}", name=f"agg{t}")
                  for t in range(3)]
        x_sb = [big.tile([128, m.tpad], F16, tag=f"x{i}", name=f"x{i}")
                for i in range(2)]
        pre0 = big.tile([128, m.tpad], F16, tag="pre0", name="pre0")
        preK = (big.tile([128, m.tpad], F16, tag="preK", name="preK")
                if m.two_pre else pre0)

        ident = S["ident"]
        iota4 = S["iota4"]

        def layernorm_batch(pz, nt_sub, y_out, b3rep):
            """nt_sub token-major [128,128] psum slabs -> normalized SBUF.

            pz: [128, nt_sub*128] psum (token-major subtiles side by side)
            y_out: [128, nt_sub*128] fp16 SBUF destination

            Drains z to fp16 SBUF first (1x psum pass), with a 132-stride
            subtile layout so bn_stats keeps its 3D subgroup structure; all
            remaining LayerNorm work runs at SBUF fp16 rates, with the
            normalize ops split between gpsimd and vector.
            """
            zsb = zpool.tile([128, 4, 132], F16, tag="zsb",
                             name="zsb")[:, :nt_sub, :]
            nc.any.tensor_copy(out=zsb[:, :, 0:128],
                               in_=pz.rearrange("p (i f) -> p i f", f=128))
            if b3rep is not None:
                nc.vector.tensor_tensor(
                    out=zsb[:, :, 0:128], in0=zsb[:, :, 0:128],
                    in1=b3rep[:, :nt_sub * 128]
                        .rearrange("p (i f) -> p i f", f=128),
                    op=ALU.add)
            st = stp.tile([128, 4, 8], F32, tag="st", name="st")
            for i in range(nt_sub):
                nc.vector.bn_stats(out=st[:, i, 0:6], in_=zsb[:, i, 0:128])
            mv = stp.tile([128, 4, 2], F32, tag="mv", name="mv")[:, :nt_sub, :]
            for i in range(nt_sub):
                nc.vector.bn_aggr(out=mv[:, i, :], in_=st[:, i, 0:6])
            sd = stp.tile([128, 4], F32, tag="sd", name="sd")[:, :nt_sub]
            nc.scalar.activation(out=sd, in_=mv[:, :, 1], func=AF.Sqrt,
                                 bias=eps_sb[:], scale=1.0)
            rstd = stp.tile([128, 4], F32, tag="rstd", name="rstd")[:, :nt_sub]
            nc.vector.reciprocal(out=rstd, in_=sd)
            nmr = stp.tile([128, 4], F32, tag="nmr", name="nmr")[:, :nt_sub]
            nc.vector.scalar_tensor_tensor(out=nmr, in0=mv[:, :, 0],
                                           scalar=-1.0, in1=rstd,
                                           op0=ALU.mult, op1=ALU.mult)
            for i in range(nt_sub):
                eng = nc.gpsimd if i % 2 == 0 else nc.vector
                eng.tensor_scalar(
                    out=y_out[:, i * 128:(i + 1) * 128],
                    in0=zsb[:, i, 0:128],
                    scalar1=rstd[:, i:i + 1], scalar2=nmr[:, i:i + 1],
                    op0=ALU.mult, op1=ALU.add)

        # ---------------- Stage 1: edge encoders + aggregation ----------
        for t in range(3):
            w1T, w2T, w3T = S[f"enc{t}_w1T"], S[f"enc{t}_w2T"], S[f"enc{t}_w3T"]
            b1, b2 = S[f"enc{t}_b1"], S[f"enc{t}_b2"]
            b3rep = S.get(f"enc{t}_b3rep") if m.has_b3[f"enc{t}"] else None
            for g in range(m.ngrp):
                base = g * m.slots_g
                ft = fpool.tile([m.esz, m.slots_g], F16, tag="ft", name="ft")
                nc.sync.dma_start(out=ft[:],
                                  in_=D[f"ef{t}"][:, base:base + m.slots_g])
                h2 = hpool.tile([128, m.slots_g], F16, tag="eh2", name="eh2")
                for sc, wd in _chunks(m.slots_g):
                    p1 = pbig.tile([128, 512], F32, tag="pb", name="pb")[:, :wd]
                    nc.tensor.matmul(p1, lhsT=w1T[:], rhs=ft[:, sc:sc + wd],
                                     start=True, stop=True)
                    h1 = hpool.tile([128, 512], F16, tag="eh1",
                                    name="eh1")[:, :wd]
                    nc.scalar.activation(out=h1, in_=p1, func=AF.Relu,
                                         bias=b1[:], scale=1.0)
                    p2 = pbig.tile([128, 512], F32, tag="pb", name="pb")[:, :wd]
                    nc.tensor.matmul(p2, lhsT=w2T[:], rhs=h1,
                                     start=True, stop=True)
                    nc.scalar.activation(out=h2[:, sc:sc + wd], in_=p2,
                                         func=AF.Relu, bias=b2[:], scale=1.0)
                sel4 = selp.tile([128, m.t_g, GSEG], F16, tag="sel", name="sel")
                nc.vector.tensor_tensor(
                    out=sel4[:],
                    in0=lidx_sb[t][:, g * m.t_g:(g + 1) * m.t_g, None]
                        .to_broadcast([128, m.t_g, GSEG]),
                    in1=iota4[:], op=ALU.is_equal)
                pg = pagg.tile([128, GSEG], F32, tag="pagg", name="pagg")
                for tb in range(0, m.t_g, 4):
                    nt_sub = min(4, m.t_g - tb)
                    sw = nt_sub * 128
                    pz = pzp.tile([128, 512], F32, tag="pz", name="pz")[:, :sw]
                    for i in range(nt_sub):
                        j = tb + i
                        nc.tensor.matmul(pz[:, i * 128:(i + 1) * 128],
                                         lhsT=h2[:, j * 128:(j + 1) * 128],
                                         rhs=w3T[:], start=True, stop=True)
                    lat = latp.tile([128, 512], F16, tag="lat",
                                    name="lat")[:, :sw]
                    layernorm_batch(pz, nt_sub, lat, b3rep)
                    for i in range(nt_sub):
                        j = tb + i
                        nc.tensor.matmul(pg[:],
                                         lhsT=lat[:, i * 128:(i + 1) * 128],
                                         rhs=sel4[:, j, :],
                                         start=(j == 0),
                                         stop=(j == m.t_g - 1))
                nc.any.tensor_copy(out=agg_sb[t][:, g * GSEG:(g + 1) * GSEG],
                                   in_=pg[:])

        # ---------------- Stage 2: pre-projection of aggregation --------
        for c0, wd in _chunks(m.tpad):
            p = pbig.tile([128, 512], F32, tag="pb", name="pb")[:, :wd]
            for t in range(3):
                nc.tensor.matmul(p, lhsT=S[f"preW{t}"][:],
                                 rhs=agg_sb[t][:, c0:c0 + wd],
                                 start=(t == 0),
                                 stop=(t == 2 and not m.has_ebeta))
            if m.has_ebeta:
                nc.tensor.matmul(p, lhsT=S["betaW"][:],
                                 rhs=cnt_sb[:, c0:c0 + wd],
                                 start=False, stop=True)
            nc.scalar.activation(out=pre0[:, c0:c0 + wd], in_=p,
                                 func=AF.Identity, bias=S["b1pre_n"][:],
                                 scale=1.0)
            if m.two_pre:
                nc.scalar.activation(out=preK[:, c0:c0 + wd], in_=p,
                                     func=AF.Identity, bias=S["b1pre_u"][:],
                                     scale=1.0)

        # ---------------- Stage 3: node encoder -> x0 (raw-normalized) --
        nb3 = S.get("node_b3rep") if m.has_b3["node"] else None
        for c0, wd in _chunks(m.tpad):
            p1 = pbig.tile([128, 512], F32, tag="pb", name="pb")[:, :wd]
            nc.tensor.matmul(p1, lhsT=S["node_w1T"][:],
                             rhs=nodeT_sb[:, c0:c0 + wd], start=True, stop=True)
            h1 = hpool.tile([128, 512], F16, tag="nh1", name="nh1")[:, :wd]
            nc.scalar.activation(out=h1, in_=p1, func=AF.Relu,
                                 bias=S["node_b1"][:], scale=1.0)
            p2 = pbig.tile([128, 512], F32, tag="pb", name="pb")[:, :wd]
            nc.tensor.matmul(p2, lhsT=S["node_w2T"][:], rhs=h1,
                             start=True, stop=True)
            h2 = hpool.tile([128, 512], F16, tag="nh2", name="nh2")[:, :wd]
            nc.scalar.activation(out=h2, in_=p2, func=AF.Relu,
                                 bias=S["node_b2"][:], scale=1.0)
            nt_sub = wd // 128
            pz = pzp.tile([128, 512], F32, tag="pz", name="pz")[:, :wd]
            for i in range(nt_sub):
                nc.tensor.matmul(pz[:, i * 128:(i + 1) * 128],
                                 lhsT=h2[:, i * 128:(i + 1) * 128],
                                 rhs=S["node_w3T"][:], start=True, stop=True)
            y4 = latp.tile([128, 512], F16, tag="y4", name="y4")[:, :wd]
            layernorm_batch(pz, nt_sub, y4, nb3)
            ptr = ptrp.tile([128, 512], F16, tag="ptr", name="ptr")[:, :wd]
            for i in range(nt_sub):
                nc.tensor.transpose(ptr[:, i * 128:(i + 1) * 128],
                                    y4[:, i * 128:(i + 1) * 128], ident[:])
            nc.any.tensor_copy(out=x_sb[0][:, c0:c0 + wd], in_=ptr)

        # ---------------- Stage 4: message-passing loop ------------------
        ub3 = S.get("upd_b3rep") if m.has_b3["upd"] else None
        for s in range(10):
            xin, xout = x_sb[s % 2], x_sb[(s + 1) % 2]
            w1gT = S["upd_w1gTn"] if s == 0 else S["upd_w1gTu"]
            pre_x = pre0 if s == 0 else preK
            for c0, wd in _chunks(m.tpad):
                p1 = pbig.tile([128, 512], F32, tag="pb", name="pb")[:, :wd]
                nc.tensor.matmul(p1, lhsT=ident[:], rhs=pre_x[:, c0:c0 + wd],
                                 start=True, stop=False)
                nc.tensor.matmul(p1, lhsT=w1gT[:], rhs=xin[:, c0:c0 + wd],
                                 start=False, stop=True)
                h1 = hpool.tile([128, 512], F16, tag="mh1", name="mh1")[:, :wd]
                nc.scalar.activation(out=h1, in_=p1, func=AF.Relu)
                p2 = pbig.tile([128, 512], F32, tag="pb", name="pb")[:, :wd]
                nc.tensor.matmul(p2, lhsT=S["upd_w2T"][:], rhs=h1,
                                 start=True, stop=True)
                h2 = hpool.tile([128, 512], F16, tag="mh2", name="mh2")[:, :wd]
                nc.scalar.activation(out=h2, in_=p2, func=AF.Relu,
                                     bias=S["upd_b2"][:], scale=1.0)
                nt_sub = wd // 128
                pz = pzp.tile([128, 512], F32, tag="pz", name="pz")[:, :wd]
                for i in range(nt_sub):
                    nc.tensor.matmul(pz[:, i * 128:(i + 1) * 128],
                                     lhsT=h2[:, i * 128:(i + 1) * 128],
                                     rhs=S["upd_w3T"][:], start=True, stop=True)
                y4 = latp.tile([128, 512], F16, tag="y4", name="y4")[:, :wd]
                layernorm_batch(pz, nt_sub, y4, ub3)
                ptr = ptrp.tile([128, 512], F16, tag="ptr", name="ptr")[:, :wd]
                for i in range(nt_sub):
                    nc.tensor.transpose(ptr[:, i * 128:(i + 1) * 128],
                                        y4[:, i * 128:(i + 1) * 128], ident[:])
                nc.any.tensor_copy(out=xout[:, c0:c0 + wd], in_=ptr)

        # ---------------- Stage 5: decoder ------------------------------
        xf = x_sb[0]
        for c0, wd in _chunks(m.tpad):
            p1 = pbig.tile([128, 512], F32, tag="pb", name="pb")[:, :wd]
            nc.tensor.matmul(p1, lhsT=S["dec_w1T"][:], rhs=xf[:, c0:c0 + wd],
                             start=True, stop=True)
            h1 = hpool.tile([128, 512], F16, tag="dh1", name="dh1")[:, :wd]
            nc.scalar.activation(out=h1, in_=p1, func=AF.Relu,
                                 bias=S["dec_b1"][:], scale=1.0)
            p2 = pbig.tile([128, 512], F32, tag="pb", name="pb")[:, :wd]
            nc.tensor.matmul(p2, lhsT=S["dec_w2T"][:], rhs=h1,
                             start=True, stop=True)
            h2 = hpool.tile([128, 512], F16, tag="dh2", name="dh2")[:, :wd]
            nc.scalar.activation(out=h2, in_=p2, func=AF.Relu,
                                 bias=S["dec_b2"][:], scale=1.0)
            p3 = pbig.tile([128, 512], F32, tag="pb",
                           name="pb")[:m.out_sz, :wd]
            nc.tensor.matmul(p3, lhsT=S["dec_w3T"][:], rhs=h2,
                             start=True, stop=True)
            ot = outp.tile([m.out_sz, 512], F32, tag="ot", name="ot")[:, :wd]
            nc.scalar.activation(out=ot, in_=p3, func=AF.Identity,
                                 bias=S["dec_b3"][:], scale=1.0)
            nc.sync.dma_start(out=out_ap[:, c0:c0 + wd], in_=ot)

    nc.compile()
    return nc


def make_in_maps(m):
    maps = []
    for c in range(m.n_cores):
        d = dict(m.core_data[c])
        for t in range(3):
            d.pop(f"cntseg{t}", None)
        if m.has_ebeta:
            d["cnt3"] = np.stack(
                [m.core_data[c][f"cntseg{t}"] for t in range(3)])
        d.update(m.weights)
        maps.append(d)
    return maps


def kernel(**inputs):
    m = prepare_host(inputs)
    nc = build_program(m)
    maps = make_in_maps(m)
    res = run_bass_kernel_spmd(nc, maps, core_ids=list(range(m.n_cores)))
    out = np.empty((m.N, m.out_sz), np.float32)
    for c in range(m.n_cores):
        out[c * m.n_loc:(c + 1) * m.n_loc] = \
            res.results[c]["outT"][:, :m.n_loc].T
    return out


# revision 19
# speedup vs baseline: 1.7135x; 1.0200x over previous
"""Trainium2 Bass kernel for the CustomGNN message-passing network.

Strategy (node-parallel across 8 NeuronCores, no collectives needed):
  - `senders` is unused by the reference network and edge latents never
    change across the 10 MP steps, so the segment-sum aggregation is
    loop-invariant: compute it once.
  - seg_id = receiver*3 + type decomposes into 3 independent per-type
    segment sums.  Each core owns a contiguous block of N/8 nodes and
    processes exactly the edges whose receiver lands in its block, so the
    whole network (edge encoders, aggregation, node update loop, decoder)
    is embarrassingly parallel across cores.
  - On-device layout: activations are kept transposed ([feature, token]);
    every linear runs with stationary weights on the tensor engine.  The
    last linear of each MLP uses the activation tile as the stationary
    operand, producing token-major output so LayerNorm uses per-partition
    (per-token) statistics; the result returns to feature-major via DMA
    transpose.  LayerNorm gain/shift are folded into the consuming linear
    weights on the host.
  - Per-type segment sums are block matmuls: edges are sorted by receiver
    on the host and packed into groups of 64 segments with a fixed number
    of 128-edge tiles per group; a data-driven 0/1 selection matrix
    (built on gpsimd from uploaded local indices via is_equal against an
    iota) maps edge latents to segment columns accumulated in PSUM.
  - LayerNorm statistics are batched: four 128x128 token-major tiles land
    side by side in one 512-wide PSUM tile, one 3D bn_stats computes
    even/odd partial stats for all four, and cheap [128,4] vector ops
    combine them.
"""

import math
import os
import sys
import types

for _p in ("/opt/trn_rl_repo",):
    if os.path.isdir(_p) and _p not in sys.path:
        sys.path.insert(0, _p)

import numpy as np

import concourse.bass as bass
import concourse.tile as tile
from concourse import bacc, mybir
from concourse.bass_utils import run_bass_kernel_spmd

F32 = mybir.dt.float32
F16 = mybir.dt.float16
AF = mybir.ActivationFunctionType
ALU = mybir.AluOpType

N_CORES = 8
GSEG = 64  # segments per aggregation group
EPS = 1e-5


def _np(a, dt=np.float32):
    return np.asarray(a).astype(dt)


def _mlp_arrays(mlp):
    Ws = [_np(W) for W in mlp["Ws"]]
    bs = [_np(b) for b in mlp["bs"]]
    g = _np(mlp["g"]) if mlp.get("g") is not None else None
    beta = _np(mlp["beta"]) if mlp.get("beta") is not None else None
    return Ws, bs, g, beta


def _col(v):
    return np.ascontiguousarray(np.asarray(v).reshape(-1, 1).astype(np.float32))


class Meta:
    pass


def prepare_host(inputs, n_cores=N_CORES):
    """Shard + sort edges, pack per-core device buffers, prep weights."""
    m = Meta()
    node_feats = _np(inputs["node_feats"])
    feats_by_type = [_np(inputs["body_feats"]), _np(inputs["cable_feats"]),
                     _np(inputs["con_feats"])]
    recv = np.asarray(inputs["receivers"]).astype(np.int64)
    N = node_feats.shape[0]
    assert N % n_cores == 0
    n_loc = N // n_cores
    tpad = ((n_loc + 127) // 128) * 128
    ngrp = tpad // GSEG
    m.N, m.n_loc, m.tpad, m.ngrp = N, n_loc, tpad, ngrp
    m.node_sz = node_feats.shape[1]
    m.esz = feats_by_type[0].shape[1]
    m.n_cores = n_cores

    ne = [f.shape[0] for f in feats_by_type]
    off = np.cumsum([0] + ne)
    per_ct = [[None] * 3 for _ in range(n_cores)]
    maxcnt = 1
    for t in range(3):
        r_t = recv[off[t]:off[t + 1]]
        order = np.argsort(r_t, kind="stable")
        r_sorted = r_t[order]
        bounds = np.searchsorted(r_sorted, np.arange(0, N + 1, n_loc))
        for c in range(n_cores):
            sl = slice(bounds[c], bounds[c + 1])
            lr = r_sorted[sl] - c * n_loc
            idx = order[sl]
            grp = lr // GSEG
            cnt = np.bincount(grp, minlength=ngrp)
            maxcnt = max(maxcnt, int(cnt.max()) if len(cnt) else 1)
            per_ct[c][t] = (lr, idx, grp, cnt)
    t_g = (maxcnt + 127) // 128
    slots_g = t_g * 128
    e_slots = ngrp * slots_g
    m.t_g, m.slots_g, m.e_slots, m.nt = t_g, slots_g, e_slots, ngrp * t_g

    m.core_data = []
    for c in range(n_cores):
        d = {}
        for t in range(3):
            lr, idx, grp, cnt = per_ct[c][t]
            gstart = np.cumsum(cnt) - cnt
            rank = np.arange(len(lr)) - gstart[grp]
            slot = grp * slots_g + rank
            fT = np.zeros((m.esz, e_slots), np.float16)
            fT[:, slot] = feats_by_type[t][idx].T
            lidxb = np.full(e_slots, 127.0, np.float16)
            lidxb[slot] = (lr % GSEG).astype(np.float16)
            d[f"ef{t}"] = fT
            lidx_arr = lidxb.reshape(ngrp, t_g, 128)
            sel = (lidx_arr[:, :, :, None]
                   == np.arange(GSEG, dtype=np.float16)).astype(np.float16)
            d[f"sel{t}"] = np.ascontiguousarray(sel.transpose(2, 0, 1, 3))
            d[f"cntseg{t}"] = np.bincount(lr, minlength=tpad).astype(np.float16)
        nT = np.zeros((m.node_sz, tpad), np.float16)
        nT[:, :n_loc] = node_feats[c * n_loc:(c + 1) * n_loc].T
        d["nodeT"] = nT
        m.core_data.append(d)

    # ---- weights ----
    w = {}
    enc_keys = ["body_enc", "cable_enc", "con_enc"]
    uWs, ubs, ug, ubeta = _mlp_arrays(inputs["node_upd"])
    nWs, nbs, ng, nbeta = _mlp_arrays(inputs["node_enc"])
    L = uWs[-1].shape[0]
    m.L = L
    m.has_b3 = {}
    m.beta_vecs = []
    for t in range(3):
        Ws, bs, g, beta = _mlp_arrays(inputs[enc_keys[t]])
        w[f"enc{t}_w1T"] = Ws[0].T.astype(np.float16)
        w[f"enc{t}_w2T"] = Ws[1].T.astype(np.float16)
        w[f"enc{t}_w3T"] = Ws[2].T.astype(np.float16)
        w[f"enc{t}_b1"] = _col(bs[0])
        w[f"enc{t}_b2"] = _col(bs[1])
        m.has_b3[f"enc{t}"] = bool(np.any(bs[2]))
        if m.has_b3[f"enc{t}"]:
            w[f"enc{t}_b3rep"] = np.tile(bs[2].astype(np.float16), (128, 4))
        # fold edge-LN g into the pre-projection, beta via counts
        W1b_t = uWs[0][:, L * (t + 1):L * (t + 2)]
        w[f"preW{t}"] = (W1b_t.T * g[:, None]).astype(np.float16)
        m.beta_vecs.append(W1b_t @ beta)
    m.has_ebeta = bool(any(np.any(v) for v in m.beta_vecs))
    if m.has_ebeta:
        w["betaW"] = np.stack(m.beta_vecs).astype(np.float16)  # [3, 128]

    w["node_w1T"] = nWs[0].T.astype(np.float16)
    w["node_w2T"] = nWs[1].T.astype(np.float16)
    w["node_w3T"] = nWs[2].T.astype(np.float16)
    w["node_b1"] = _col(nbs[0])
    w["node_b2"] = _col(nbs[1])
    m.has_b3["node"] = bool(np.any(nbs[2]))
    if m.has_b3["node"]:
        w["node_b3rep"] = np.tile(nbs[2].astype(np.float16), (128, 4))

    # node-update MLP; L1 split into x-part (with LN folds) and agg-part
    W1a = uWs[0][:, :L]
    w["upd_w1gTn"] = (W1a.T * ng[:, None]).astype(np.float16)  # step 0
    w["upd_w1gTu"] = (W1a.T * ug[:, None]).astype(np.float16)  # steps 1-9
    w["upd_w2T"] = uWs[1].T.astype(np.float16)
    w["upd_w3T"] = uWs[2].T.astype(np.float16)
    w["b1pre_n"] = _col(ubs[0] + W1a @ nbeta)
    w["b1pre_u"] = _col(ubs[0] + W1a @ ubeta)
    w["upd_b2"] = _col(ubs[1])
    m.has_b3["upd"] = bool(np.any(ubs[2]))
    if m.has_b3["upd"]:
        w["upd_b3rep"] = np.tile(ubs[2].astype(np.float16), (128, 4))

    dWs, dbs, _, _ = _mlp_arrays(inputs["dec"])
    w["dec_w1T"] = (dWs[0].T * ug[:, None]).astype(np.float16)
    w["dec_w2T"] = dWs[1].T.astype(np.float16)
    w["dec_w3T"] = dWs[2].T.astype(np.float16)
    w["dec_b1"] = _col(dbs[0] + dWs[0] @ ubeta)
    w["dec_b2"] = _col(dbs[1])
    w["dec_b3"] = _col(dbs[2])
    m.out_sz = dWs[2].shape[0]

    w["ident"] = np.eye(128, dtype=np.float16)
    m.weights = w
    m.two_pre = bool(np.any(w["b1pre_n"] != w["b1pre_u"])
                     or np.any(w["upd_w1gTn"] != w["upd_w1gTu"]))
    return m


def _chunks(total, step=512):
    out = []
    c = 0
    while c < total:
        out.append((c, min(step, total - c)))
        c += step
    return out


def build_program(m):
    nc = bacc.Bacc("TRN2", target_bir_lowering=False, debug=False)
    D = {}

    def din(name, shape, dt):
        D[name] = nc.dram_tensor(name, list(shape), dt, kind="ExternalInput").ap()

    for t in range(3):
        din(f"ef{t}", (m.esz, m.e_slots), F16)
        din(f"sel{t}", (128, m.ngrp, m.t_g, GSEG), F16)
    if m.has_ebeta:
        din("cnt3", (3, m.tpad), F16)
    din("nodeT", (m.node_sz, m.tpad), F16)
    for k, v in m.weights.items():
        din(k, v.shape, F16 if v.dtype == np.float16 else F32)
    out_ap = nc.dram_tensor("outT", [m.out_sz, m.tpad], F32,
                            kind="ExternalOutput").ap()

    from contextlib import ExitStack
    with tile.TileContext(nc) as tc, ExitStack() as ctx:
        sing = ctx.enter_context(tc.tile_pool(name="sing", bufs=1))
        big = ctx.enter_context(tc.tile_pool(name="big", bufs=1))
        fpool = ctx.enter_context(tc.tile_pool(name="fpool", bufs=3))
        hpool = ctx.enter_context(tc.tile_pool(name="hpool", bufs=3))
        latp = ctx.enter_context(tc.tile_pool(name="latp", bufs=3))
        selp = ctx.enter_context(tc.tile_pool(name="selp", bufs=3))
        stp = ctx.enter_context(tc.tile_pool(name="stp", bufs=6))
        zpool = ctx.enter_context(tc.tile_pool(name="zpool", bufs=3))
        outp = ctx.enter_context(tc.tile_pool(name="outp", bufs=3))
        pbig = ctx.enter_context(tc.tile_pool(name="pbig", bufs=2, space="PSUM"))
        pzp = ctx.enter_context(tc.tile_pool(name="pzp", bufs=2, space="PSUM"))
        pagg = ctx.enter_context(tc.tile_pool(name="pagg", bufs=2, space="PSUM"))
        ptrp = ctx.enter_context(tc.tile_pool(name="ptrp", bufs=2, space="PSUM"))

        # resident SBUF tensors
        S = {}
        for k, v in m.weights.items():
            dt = F16 if v.dtype == np.float16 else F32
            S[k] = sing.tile(list(v.shape), dt, name=k, tag=k)
            nc.sync.dma_start(out=S[k][:], in_=D[k][:])
        if m.has_ebeta:
            cnt_sb = sing.tile([3, m.tpad], F16, name="cnt_sb", tag="cnt_sb")
            nc.sync.dma_start(out=cnt_sb[:], in_=D["cnt3"][:])
        nodeT_sb = sing.tile([m.node_sz, m.tpad], F16, name="nodeT_sb",
                             tag="nodeT_sb")
        nc.sync.dma_start(out=nodeT_sb[:], in_=D["nodeT"][:])
        eps_sb = sing.tile([128, 1], F32, name="eps_sb", tag="eps_sb")
        nc.vector.memset(eps_sb[:], EPS)

        agg_sb = [big.tile([128, m.tpad], F16, tag=f"agg{t}", name=f"agg{t}")
                  for t in range(3)]
        x_sb = [big.tile([128, m.tpad], F16, tag=f"x{i}", name=f"x{i}")
                for i in range(2)]
        pre0 = big.tile([128, m.tpad], F16, tag="pre0", name="pre0")
        preK = (big.tile([128, m.tpad], F16, tag="preK", name="preK")
                if m.two_pre else pre0)

        ident = S["ident"]

        def layernorm_batch(pz, nt_sub, y_out, b3rep):
            """nt_sub token-major [128,128] psum slabs -> normalized SBUF.

            pz: [128, nt_sub*128] psum (token-major subtiles side by side)
            y_out: [128, nt_sub*128] fp16 SBUF destination

            The psum drain writes pairs of subtiles element-interleaved into
            SBUF, so one contiguous bn_stats call per pair returns both
            tiles' mean and M2 as its even/odd statistics (no bn_aggr);
            var/128 folds into the Sqrt scale.  Normalize ops are rotated
            across vector/gpsimd/scalar.
            """
            if b3rep is not None:
                nc.vector.tensor_tensor(out=pz[:], in0=pz[:],
                                        in1=b3rep[:, :nt_sub * 128],
                                        op=ALU.add)
            npair = nt_sub // 2
            odd = nt_sub % 2
            zsb = zpool.tile([128, 2, 256], F16, tag="zsb", name="zsb")
            if npair:
                nc.any.tensor_copy(
                    out=zsb[:, 0:npair, :].rearrange("p a (f b) -> p a b f",
                                                     b=2),
                    in_=pz[:, 0:npair * 256].rearrange("p (a b f) -> p a b f",
                                                       b=2, f=128))
            st = stp.tile([128, 3, 8], F32, tag="st", name="st")
            for j in range(npair):
                nc.vector.bn_stats(out=st[:, j, 0:6], in_=zsb[:, j, :])
            if odd:
                nc.vector.bn_stats(out=st[:, npair, 0:6],
                                   in_=pz[:, npair * 256:npair * 256 + 128])
                mvx = stp.tile([128, 2], F32, tag="mvx", name="mvx")
                nc.vector.bn_aggr(out=mvx[:], in_=st[:, npair, 0:6])
                sdx = stp.tile([128, 1], F32, tag="sdx", name="sdx")
                nc.scalar.activation(out=sdx[:], in_=mvx[:, 1:2], func=AF.Sqrt,
                                     bias=eps_sb[:], scale=1.0)
                rstdx = stp.tile([128, 1], F32, tag="rstdx", name="rstdx")
                nc.vector.reciprocal(out=rstdx[:], in_=sdx[:])
                nmrx = stp.tile([128, 1], F32, tag="nmrx", name="nmrx")
                nc.vector.scalar_tensor_tensor(out=nmrx[:], in0=mvx[:, 0:1],
                                               scalar=-1.0, in1=rstdx[:],
                                               op0=ALU.mult, op1=ALU.mult)
            if npair:
                meanv = st[:, 0:npair, 1:5:3]   # [128, npair, 2]
                m2v = st[:, 0:npair, 2:6:3]     # [128, npair, 2]
                sd = stp.tile([128, 3, 2], F32, tag="sd",
                              name="sd")[:, 0:npair, :]
                # sqrt(M2/128 + eps) : even/odd halves are full tiles of 128
                nc.scalar.activation(out=sd, in_=m2v, func=AF.Sqrt,
                                     bias=eps_sb[:], scale=1.0 / 128)
                rstd = stp.tile([128, 3, 2], F32, tag="rstd",
                                name="rstd")[:, 0:npair, :]
                nc.vector.reciprocal(out=rstd, in_=sd)
                nmr = stp.tile([128, 3, 2], F32, tag="nmr",
                               name="nmr")[:, 0:npair, :]
                nc.vector.scalar_tensor_tensor(out=nmr, in0=meanv,
                                               scalar=-1.0, in1=rstd,
                                               op0=ALU.mult, op1=ALU.mult)
            engs = [nc.vector, nc.gpsimd, nc.scalar, nc.gpsimd]
            for i in range(nt_sub):
                if i < 2 * npair:
                    r1 = rstd[:, i // 2, i % 2:i % 2 + 1]
                    n1 = nmr[:, i // 2, i % 2:i % 2 + 1]
                    src_ap = zsb[:, i // 2, i % 2:256:2]
                else:
                    r1, n1 = rstdx[:], nmrx[:]
                    src_ap = pz[:, i * 128:(i + 1) * 128]
                eng = engs[i % 4]
                if eng is nc.scalar:
                    nc.scalar.activation(
                        out=y_out[:, i * 128:(i + 1) * 128],
                        in_=src_ap, func=AF.Identity, bias=n1, scale=r1)
                elif eng is nc.gpsimd and i >= 2 * npair:
                    # gpsimd cannot read psum; odd trailing tile -> vector
                    nc.vector.tensor_scalar(
                        out=y_out[:, i * 128:(i + 1) * 128], in0=src_ap,
                        scalar1=r1, scalar2=n1, op0=ALU.mult, op1=ALU.add)
                else:
                    eng.tensor_scalar(
                        out=y_out[:, i * 128:(i + 1) * 128], in0=src_ap,
                        scalar1=r1, scalar2=n1, op0=ALU.mult, op1=ALU.add)

        # ---------------- Stage 1: edge encoders + aggregation ----------
        for t in range(3):
            w1T, w2T, w3T = S[f"enc{t}_w1T"], S[f"enc{t}_w2T"], S[f"enc{t}_w3T"]
            b1, b2 = S[f"enc{t}_b1"], S[f"enc{t}_b2"]
            b3rep = S.get(f"enc{t}_b3rep") if m.has_b3[f"enc{t}"] else None
            for g in range(m.ngrp):
                base = g * m.slots_g
                ft = fpool.tile([m.esz, m.slots_g], F16, tag="ft", name="ft")
                nc.sync.dma_start(out=ft[:],
                                  in_=D[f"ef{t}"][:, base:base + m.slots_g])
                h2 = hpool.tile([128, m.slots_g], F16, tag="eh2", name="eh2")
                for sc, wd in _chunks(m.slots_g):
                    p1 = pbig.tile([128, 512], F32, tag="pb", name="pb")[:, :wd]
                    nc.tensor.matmul(p1, lhsT=w1T[:], rhs=ft[:, sc:sc + wd],
                                     start=True, stop=True)
                    h1 = hpool.tile([128, 512], F16, tag="eh1",
                                    name="eh1")[:, :wd]
                    nc.scalar.activation(out=h1, in_=p1, func=AF.Relu,
                                         bias=b1[:], scale=1.0)
                    p2 = pbig.tile([128, 512], F32, tag="pb", name="pb")[:, :wd]
                    nc.tensor.matmul(p2, lhsT=w2T[:], rhs=h1,
                                     start=True, stop=True)
                    nc.scalar.activation(out=h2[:, sc:sc + wd], in_=p2,
                                         func=AF.Relu, bias=b2[:], scale=1.0)
                sel4 = selp.tile([128, m.t_g, GSEG], F16, tag="sel", name="sel")
                nc.sync.dma_start(out=sel4[:], in_=D[f"sel{t}"][:, g, :, :])
                pg = pagg.tile([128, GSEG], F32, tag="pagg", name="pagg")
                for tb in range(0, m.t_g, 4):
                    nt_sub = min(4, m.t_g - tb)
                    sw = nt_sub * 128
                    pz = pzp.tile([128, 512], F32, tag="pz", name="pz")[:, :sw]
                    for i in range(nt_sub):
                        j = tb + i
                        nc.tensor.matmul(pz[:, i * 128:(i + 1) * 128],
                                         lhsT=h2[:, j * 128:(j + 1) * 128],
                                         rhs=w3T[:], start=True, stop=True)
                    lat = latp.tile([128, 512], F16, tag="lat",
                                    name="lat")[:, :sw]
                    layernorm_batch(pz, nt_sub, lat, b3rep)
                    for i in range(nt_sub):
                        j = tb + i
                        nc.tensor.matmul(pg[:],
                                         lhsT=lat[:, i * 128:(i + 1) * 128],
                                         rhs=sel4[:, j, :],
                                         start=(j == 0),
                                         stop=(j == m.t_g - 1))
                nc.scalar.activation(out=agg_sb[t][:, g * GSEG:(g + 1) * GSEG],
                                     in_=pg[:], func=AF.Identity)

        # ---------------- Stage 2: pre-projection of aggregation --------
        for c0, wd in _chunks(m.tpad):
            p = pbig.tile([128, 512], F32, tag="pb", name="pb")[:, :wd]
            for t in range(3):
                nc.tensor.matmul(p, lhsT=S[f"preW{t}"][:],
                                 rhs=agg_sb[t][:, c0:c0 + wd],
                                 start=(t == 0),
                                 stop=(t == 2 and not m.has_ebeta))
            if m.has_ebeta:
                nc.tensor.matmul(p, lhsT=S["betaW"][:],
                                 rhs=cnt_sb[:, c0:c0 + wd],
                                 start=False, stop=True)
            nc.scalar.activation(out=pre0[:, c0:c0 + wd], in_=p,
                                 func=AF.Identity, bias=S["b1pre_n"][:],
                                 scale=1.0)
            if m.two_pre:
                nc.scalar.activation(out=preK[:, c0:c0 + wd], in_=p,
                                     func=AF.Identity, bias=S["b1pre_u"][:],
                                     scale=1.0)

        # ---------------- Stage 3: node encoder -> x0 (raw-normalized) --
        nb3 = S.get("node_b3rep") if m.has_b3["node"] else None
        for c0, wd in _chunks(m.tpad):
            p1 = pbig.tile([128, 512], F32, tag="pb", name="pb")[:, :wd]
            nc.tensor.matmul(p1, lhsT=S["node_w1T"][:],
                             rhs=nodeT_sb[:, c0:c0 + wd], start=True, stop=True)
            h1 = hpool.tile([128, 512], F16, tag="nh1", name="nh1")[:, :wd]
            nc.scalar.activation(out=h1, in_=p1, func=AF.Relu,
                                 bias=S["node_b1"][:], scale=1.0)
            p2 = pbig.tile([128, 512], F32, tag="pb", name="pb")[:, :wd]
            nc.tensor.matmul(p2, lhsT=S["node_w2T"][:], rhs=h1,
                             start=True, stop=True)
            h2 = hpool.tile([128, 512], F16, tag="nh2", name="nh2")[:, :wd]
            nc.scalar.activation(out=h2, in_=p2, func=AF.Relu,
                                 bias=S["node_b2"][:], scale=1.0)
            nt_sub = wd // 128
            pz = pzp.tile([128, 512], F32, tag="pz", name="pz")[:, :wd]
            for i in range(nt_sub):
                nc.tensor.matmul(pz[:, i * 128:(i + 1) * 128],
                                 lhsT=h2[:, i * 128:(i + 1) * 128],
                                 rhs=S["node_w3T"][:], start=True, stop=True)
            y4 = latp.tile([128, 512], F16, tag="y4", name="y4")[:, :wd]
            layernorm_batch(pz, nt_sub, y4, nb3)
            ptr = ptrp.tile([128, 512], F16, tag="ptr", name="ptr")[:, :wd]
            for i in range(nt_sub):
                nc.tensor.transpose(ptr[:, i * 128:(i + 1) * 128],
                                    y4[:, i * 128:(i + 1) * 128], ident[:])
            nc.any.tensor_copy(out=x_sb[0][:, c0:c0 + wd], in_=ptr)

        # ---------------- Stage 4: message-passing loop ------------------
        ub3 = S.get("upd_b3rep") if m.has_b3["upd"] else None
        for s in range(10):
            xin, xout = x_sb[s % 2], x_sb[(s + 1) % 2]
            w1gT = S["upd_w1gTn"] if s == 0 else S["upd_w1gTu"]
            pre_x = pre0 if s == 0 else preK
            for c0, wd in _chunks(m.tpad):
                p1 = pbig.tile([128, 512], F32, tag="pb", name="pb")[:, :wd]
                nc.tensor.matmul(p1, lhsT=ident[:], rhs=pre_x[:, c0:c0 + wd],
                                 start=True, stop=False)
                nc.tensor.matmul(p1, lhsT=w1gT[:], rhs=xin[:, c0:c0 + wd],
                                 start=False, stop=True)
                h1 = hpool.tile([128, 512], F16, tag="mh1", name="mh1")[:, :wd]
                nc.scalar.activation(out=h1, in_=p1, func=AF.Relu)
                p2 = pbig.tile([128, 512], F32, tag="pb", name="pb")[:, :wd]
                nc.tensor.matmul(p2, lhsT=S["upd_w2T"][:], rhs=h1,
                                 start=True, stop=True)
                h2 = hpool.tile([128, 512], F16, tag="mh2", name="mh2")[:, :wd]
                nc.scalar.activation(out=h2, in_=p2, func=AF.Relu,
                                     bias=S["upd_b2"][:], scale=1.0)
                nt_sub = wd // 128
                pz = pzp.tile([128, 512], F32, tag="pz", name="pz")[:, :wd]
                for i in range(nt_sub):
                    nc.tensor.matmul(pz[:, i * 128:(i + 1) * 128],
                                     lhsT=h2[:, i * 128:(i + 1) * 128],
                                     rhs=S["upd_w3T"][:], start=True, stop=True)
                y4 = latp.tile([128, 512], F16, tag="y4", name="y4")[:, :wd]
                layernorm_batch(pz, nt_sub, y4, ub3)
                ptr = ptrp.tile([128, 512], F16, tag="ptr", name="ptr")[:, :wd]
                for i in range(nt_sub):
                    nc.tensor.transpose(ptr[:, i * 128:(i + 1) * 128],
                                        y4[:, i * 128:(i + 1) * 128], ident[:])
                nc.any.tensor_copy(out=xout[:, c0:c0 + wd], in_=ptr)

        # ---------------- Stage 5: decoder ------------------------------
        xf = x_sb[0]
        for c0, wd in _chunks(m.tpad):
            p1 = pbig.tile([128, 512], F32, tag="pb", name="pb")[:, :wd]
            nc.tensor.matmul(p1, lhsT=S["dec_w1T"][:], rhs=xf[:, c0:c0 + wd],
                             start=True, stop=True)
            h1 = hpool.tile([128, 512], F16, tag="dh1", name="dh1")[:, :wd]
            nc.scalar.activation(out=h1, in_=p1, func=AF.Relu,
                                 bias=S["dec_b1"][:], scale=1.0)
            p2 = pbig.tile([128, 512], F32, tag="pb", name="pb")[:, :wd]
            nc.tensor.matmul(p2, lhsT=S["dec_w2T"][:], rhs=h1,
                             start=True, stop=True)
            h2 = hpool.tile([128, 512], F16, tag="dh2", name="dh2")[:, :wd]
            nc.scalar.activation(out=h2, in_=p2, func=AF.Relu,
                                 bias=S["dec_b2"][:], scale=1.0)
            p3 = pbig.tile([128, 512], F32, tag="pb",
                           name="pb")[:m.out_sz, :wd]
            nc.tensor.matmul(p3, lhsT=S["dec_w3T"][:], rhs=h2,
                             start=True, stop=True)
            ot = outp.tile([m.out_sz, 512], F32, tag="ot", name="ot")[:, :wd]
            nc.scalar.activation(out=ot, in_=p3, func=AF.Identity,
                                 bias=S["dec_b3"][:], scale=1.0)
            nc.sync.dma_start(out=out_ap[:, c0:c0 + wd], in_=ot)

    nc.compile()
    return nc


def make_in_maps(m):
    maps = []
    for c in range(m.n_cores):
        d = dict(m.core_data[c])
        for t in range(3):
            d.pop(f"cntseg{t}", None)
        if m.has_ebeta:
            d["cnt3"] = np.stack(
                [m.core_data[c][f"cntseg{t}"] for t in range(3)])
        d.update(m.weights)
        maps.append(d)
    return maps


def kernel(**inputs):
    m = prepare_host(inputs)
    nc = build_program(m)
    maps = make_in_maps(m)
    res = run_bass_kernel_spmd(nc, maps, core_ids=list(range(m.n_cores)))
    out = np.empty((m.N, m.out_sz), np.float32)
    for c in range(m.n_cores):
        out[c * m.n_loc:(c + 1) * m.n_loc] = \
            res.results[c]["outT"][:, :m.n_loc].T
    return out
